# revision 34
# baseline (speedup 1.0000x reference)
"""AttentionPoolHead Trainium2 kernel (8 NeuronCores, batch-data-parallel).

Takes FULL inputs (as produced by setup_inputs), returns FULL (B, C) output.

Math (exact rewrite of the reference):
  tokens = [patches | cls | storage | zero-pad]            (order irrelevant: softmax-pool
                                                            is permutation invariant)
  kv     = LN(tokens) ; q fixed vector  =>  per-token score collapses to
      s[t,h] = r_t * (x_t . w''[:,h])
  with w'' = (Wk_head.T @ qp) * g / sqrt(HD), mean-centered over D (the -mu term of LN
  and all per-head constants vanish under softmax).  With q_t = p_t * r_t,
  p_t = exp(s_t):
      head_mix_h = ((sum_t q_t x_t) - rowmean correction) / (sum_t p_t)
      ctx = Wv' @ head_mix (+ folded biases), pooled = Wo @ ctx + bo', LN, Wp'' head.

Pipeline notes (v2):
  - token stream is read twice: bf16 natural layout (mix matmul rhs) and f8
    transposed per-super-contiguous layout (score matmul stationary).
  - per-token sum(x^2) is split across ACT (4/8), DVE (2/8), GPSIMD (2/8).
  - PE emission is software-pipelined one super: scores(s) ; mix(s-1), so the
    DVE score->qp chain of super s-1 overlaps the score matmuls of super s.
  - the per-core tail runs twice on batch halves (0-3 after batch 3, 4-7 at
    the end) so most of it hides under the second half's token streaming.
"""

import numpy as np

B, S, N, D, H, C = 64, 4, 4096, 1024, 16, 14
HD = D // H
EPS = 1e-5
NCORES = 8
BLOC = B // NCORES          # batches per core
NREAL = 1 + S + N           # 4101 real tokens
NTAIL = 8                   # tail mini-super: 5 real + 3 zero-pad tokens
LTOT = N + NTAIL            # 4104 tokens kept per batch
NSUPER = 4                  # full supers of 1024 tokens (the patches region)
SUPER = 1024
NSUB = 8                    # 128-token subtiles per full super

_cache = {}


def _f32(x):
    return np.ascontiguousarray(np.asarray(x, dtype=np.float32))


def _host_prep(inputs):
    """All weight folding + token layout prep on the host (numpy)."""
    import ml_dtypes

    bf16 = ml_dtypes.bfloat16
    f8 = ml_dtypes.float8_e4m3

    cls_tok = _f32(inputs["cls_tok"])        # [B, D]
    storage = _f32(inputs["storage"])        # [B, S, D]
    patches = _f32(inputs["patches"])        # [B, N, D]
    query = _f32(inputs["query"]).reshape(D)
    g_kv = _f32(inputs["ln_kv_g"])
    b_kv = _f32(inputs["ln_kv_b"])
    Wq = _f32(inputs["Wq"]); Wk = _f32(inputs["Wk"]); Wv = _f32(inputs["Wv"])
    bq = _f32(inputs["bq"])
    Wo = _f32(inputs["Wo"]); bo = _f32(inputs["bo"])
    g_out = _f32(inputs["ln_out_g"]); b_out = _f32(inputs["ln_out_b"])
    Wp = _f32(inputs["Wp"]); bp = _f32(inputs["bp"])

    # --- score weights: s[t,h] = r_t * (x_t . w''[:, h]) ----------------------
    qp = query @ Wq.T + bq                                   # [D]
    w_raw = np.einsum("hid,hi->dh", Wk.reshape(H, HD, D), qp.reshape(H, HD))
    w_raw /= np.sqrt(HD).astype(np.float32)
    wpr = w_raw * g_kv[:, None]                              # fold LN gain
    wpp = wpr - wpr.mean(0, keepdims=True)                   # fold LN mean-centering
    # device layout [128, 8, 17]: [:, c, 0:16] = wpp[128c+p, :], col 16 = ones
    wsc_dev = np.zeros((128, 8, 17), dtype=np.float32)
    wsc_dev[:, :, :16] = wpp.reshape(8, 128, 16).transpose(1, 0, 2)
    wsc_dev[:, :, 16] = 1.0
    wsc_dev = wsc_dev.astype(bf16)

    # --- Wv / Wo / Wp folds ---------------------------------------------------
    WvT = (Wv * g_kv[None, :]).T                             # [D_in, D_out]
    wvT_dev = np.ascontiguousarray(
        WvT.reshape(8, 128, D).transpose(1, 0, 2)).astype(bf16)   # [128, 8, 1024]
    woT_dev = np.ascontiguousarray(
        Wo.T.reshape(8, 128, D).transpose(1, 0, 2)).astype(bf16)  # [128, 8, 1024]
    WpT = (Wp * g_out[None, :]).T                            # [D, C]
    wpT_dev = np.ascontiguousarray(
        WpT.reshape(8, 128, C).transpose(1, 0, 2)).astype(bf16)   # [128, 8, 14]

    bo_comb = bo + Wo @ (Wv @ b_kv)                          # [D]
    bo_dev = np.ascontiguousarray(
        bo_comb.reshape(8, 128).T).astype(np.float32)        # [128, 8] = boT
    bp_comb = (bp + Wp @ b_out).reshape(C, 1).astype(np.float32)  # [14, 1]

    # --- token stream in both layouts ----------------------------------------
    # natural: order patches (0..N-1), cls, storage, pad
    tok = np.zeros((B, LTOT, D), dtype=bf16)
    tok[:, :N] = patches.astype(bf16)
    tok[:, N] = cls_tok.astype(bf16)
    tok[:, N + 1:N + 1 + S] = storage.astype(bf16)
    # p-major per-super natural layout (16 KB contiguous per partition, so
    # DMA descriptors are large and the nat queue keeps pace with the tt one):
    #   tok_main[b, s, p, j, d] = tok[b, 1024 s + 128 j + p, d]
    tok_main = np.ascontiguousarray(
        tok[:, :N].reshape(B, NSUPER, 8, 128, D).transpose(0, 1, 3, 2, 4))
    tok_tail = np.ascontiguousarray(tok[:, N:])              # [B, NTAIL, D]
    # transposed f8, p-major per-super contiguous:
    #   tokT_main[b, s, p, c, t] = tok[b, 1024 s + t, 128 c + p]
    tokf8 = tok[:, :N].astype(f8)                            # [B, N, D]
    tokT_main = np.ascontiguousarray(
        tokf8.reshape(B, NSUPER, SUPER, 8, 128).transpose(0, 1, 4, 3, 2))
    #   tokT_tail[b, p, c, t] = tok[b, N + t, 128 c + p]
    tokT_tail = np.ascontiguousarray(
        tok[:, N:].astype(f8).reshape(B, NTAIL, 8, 128).transpose(0, 3, 2, 1))

    weights = dict(wsc=wsc_dev, wvT=wvT_dev, woT=woT_dev, wpT=wpT_dev,
                   bo=bo_dev, bp=bp_comb)
    return tok_main, tok_tail, tokT_main, tokT_tail, weights


def _emit(tc, io):
    """Emit the Tile program for one core (BLOC batches)."""
    import concourse.bass as bass
    from concourse import mybir

    nc = tc.nc
    f32 = mybir.dt.float32
    bf16 = mybir.dt.bfloat16
    f8 = mybir.dt.float8e4
    AF = mybir.ActivationFunctionType
    OP = mybir.AluOpType

    toknatm, toknatt, tokTm, tokTt, wsc, wvT, woT, wpT, bo, bp, out = (
        io["toknatm"], io["toknatt"], io["tokTm"], io["tokTt"], io["wsc"],
        io["wvT"], io["woT"], io["wpT"], io["bo"], io["bp"], io["out"])

    from contextlib import ExitStack
    ctx = ExitStack()
    with ctx:
        singles = ctx.enter_context(tc.tile_pool(name="singles", bufs=1))
        nat_pool = ctx.enter_context(tc.tile_pool(name="nat", bufs=5))
        tt_pool = ctx.enter_context(tc.tile_pool(name="tt", bufs=5))
        tl_pool = ctx.enter_context(tc.tile_pool(name="tl", bufs=2))
        small = ctx.enter_context(tc.tile_pool(name="small", bufs=3))
        ep_pool = ctx.enter_context(tc.tile_pool(name="ep", bufs=2))
        ps_small = ctx.enter_context(tc.tile_pool(name="ps_small", bufs=2, space="PSUM"))
        ps_mix = ctx.enter_context(tc.tile_pool(name="ps_mix", bufs=2, space="PSUM"))
        ps_den = ctx.enter_context(tc.tile_pool(name="ps_den", bufs=1, space="PSUM"))
        ps_tp = ctx.enter_context(tc.tile_pool(name="ps_tp", bufs=1, space="PSUM"))

        # ---- score weights (needed from the first super) ----------------
        wsc_sb = singles.tile([128, 8, 17], bf16)
        nc.sync.dma_start(wsc_sb[:], wsc[:])
        # tail weights are DMA'd once batch 0's token loads are queued
        wvT_sb = singles.tile([128, 8, D], bf16)
        woT_sb = singles.tile([128, 8, D], bf16)
        wpT_sb = singles.tile([128, 8, C], bf16)
        bo_sb = singles.tile([128, 8], f32)      # boT[p, k2] = bo_comb[128*k2+p]
        bp_sb = singles.tile([C, 1], f32)

        from concourse.masks import make_identity
        ident_b = singles.tile([128, 128], bf16)
        make_identity(nc, ident_b[:])
        onesf = singles.tile([128, 1], f32)
        nc.vector.memset(onesf[:], 1.0)
        ones_row = singles.tile([1, 128], f32)
        nc.vector.memset(ones_row[:], 1.0)

        # persistent per-core accumulators
        mixnT_all = singles.tile([128, 8, H, BLOC], bf16)    # [dp, c, h, b]

        x2junk = singles.tile([128, 1024], bf16)
        x2junk_a = singles.tile([128, 1024], bf16)

        def load_super(b, s):
            """Issue DMAs + x^2 + scores for (b, s); return state dict."""
            if s < NSUPER:
                natT = nat_pool.tile([128, NSUB, 1024], bf16, tag="nat")
                aux = nat_pool.tile([128, NSUB, 3], bf16, tag="aux")
                ttT = tt_pool.tile([128, 8, SUPER], f8, tag="tt")
                nc.gpsimd.dma_start(ttT[:], tokTm[b, s])
                nc.sync.dma_start(natT[:], toknatm[b, s])
                nc.vector.memset(aux[:, :, 0:1], 1.0)
                nsub, npart = NSUB, 128
            else:
                natT = tl_pool.tile([NTAIL, 1, 1024], bf16, tag="natl")
                aux = tl_pool.tile([NTAIL, 1, 3], bf16, tag="auxl")
                ttT = tl_pool.tile([128, 8, NTAIL], f8, tag="ttl")
                nc.gpsimd.dma_start(ttT[:], tokTt[b])
                nc.sync.dma_start(
                    natT[:],
                    toknatt[b].rearrange("(j p) d -> p j d", p=NTAIL))
                nc.vector.memset(aux[:, :, 0:1], 1.0)
                nsub, npart = 1, NTAIL

            # per-token sum of squares, split DVE 3/8 + ACT 5/8
            x2acc = small.tile([128, 8], f32, tag="x2acc")
            for j in range(nsub):
                if s == NSUPER:
                    src = natT[:, j, 0:1024]     # [8, 1024]
                    nc.vector.scalar_tensor_tensor(
                        out=x2junk[0:npart, :], in0=src, scalar=1.0, in1=src,
                        op0=OP.mult, op1=OP.mult,
                        accum_out=x2acc[0:npart, j:j + 1])
                elif j in (0, 3, 6):
                    nc.vector.scalar_tensor_tensor(
                        out=x2junk[:], in0=natT[:, j, 0:1024], scalar=1.0,
                        in1=natT[:, j, 0:1024], op0=OP.mult, op1=OP.mult,
                        accum_out=x2acc[:, j:j + 1])
                else:
                    nc.scalar.activation(
                        x2junk_a[:], natT[:, j, 0:1024], AF.Square,
                        accum_out=x2acc[:, j:j + 1])

            # scores: s~[tok, h] (+ col16 = sum_d x) accumulated over 8 D-chunks
            scps = ps_small.tile([128, 8, 17], f32, tag="scps")
            ntok = 128 if s < NSUPER else NTAIL
            for j in range(nsub):
                for c in range(8):
                    nc.tensor.matmul(
                        scps[0:npart, j, :],
                        lhsT=ttT[:, c, 128 * j:128 * j + ntok],
                        rhs=wsc_sb[:, c, :],
                        start=(c == 0), stop=(c == 7))
            return dict(natT=natT, aux=aux, ttT=ttT, x2acc=x2acc, scps=scps,
                        nsub=nsub, npart=npart, s=s, b=b)

        def chain_super(st):
            """LN stats -> r -> softmax weights for a loaded super (DVE/ACT)."""
            nsub, npart = st["nsub"], st["npart"]
            natT, aux, x2acc, scps = (st["natT"], st["aux"], st["x2acc"],
                                      st["scps"])
            sx = small.tile([128, 8], f32, tag="sx")
            nc.vector.tensor_copy(sx[0:npart, 0:nsub], scps[0:npart, 0:nsub, 16])
            # stash mu_t = sum(x)/1024 into aux column 1 (den matmul rhs)
            nc.vector.tensor_scalar_mul(aux[:, 0:nsub, 1:2],
                                        sx[0:npart, 0:nsub], 1.0 / 1024.0)
            v0 = small.tile([128, 8], f32, tag="v0")
            nc.vector.scalar_tensor_tensor(
                out=v0[0:npart, 0:nsub],
                in0=sx[0:npart, 0:nsub], scalar=-1.0 / (1024.0 * 1024.0),
                in1=sx[0:npart, 0:nsub], op0=OP.mult, op1=OP.mult)
            nc.vector.scalar_tensor_tensor(
                out=v0[0:npart, 0:nsub],
                in0=x2acc[0:npart, 0:nsub], scalar=1.0 / 1024.0,
                in1=v0[0:npart, 0:nsub], op0=OP.mult, op1=OP.add)
            nc.vector.tensor_scalar_add(v0[0:npart, 0:nsub],
                                        v0[0:npart, 0:nsub], EPS)
            # r = rsqrt(wn) via 2 Newton steps from seed 1.0 (wn ~= 1 +- 0.07)
            rr = small.tile([128, 8], f32, tag="rr")
            ra = small.tile([128, 8], f32, tag="ra")
            rc = small.tile([128, 8], f32, tag="rc")
            nc.vector.tensor_scalar(rr[0:npart, 0:nsub], v0[0:npart, 0:nsub],
                                    -0.5, 1.5, op0=OP.mult, op1=OP.add)
            nc.vector.scalar_tensor_tensor(
                out=ra[0:npart, 0:nsub], in0=rr[0:npart, 0:nsub], scalar=1.0,
                in1=rr[0:npart, 0:nsub], op0=OP.mult, op1=OP.mult)
            nc.vector.scalar_tensor_tensor(
                out=rc[0:npart, 0:nsub], in0=v0[0:npart, 0:nsub], scalar=-0.5,
                in1=ra[0:npart, 0:nsub], op0=OP.mult, op1=OP.mult)
            nc.vector.scalar_tensor_tensor(
                out=rr[0:npart, 0:nsub], in0=rc[0:npart, 0:nsub], scalar=1.5,
                in1=rr[0:npart, 0:nsub], op0=OP.add, op1=OP.mult)
            # stash 1/r = wn * r into aux column 2: the den matmul then
            # accumulates sum_t q/r = sum_t p from the q-columns alone
            nc.vector.scalar_tensor_tensor(
                out=aux[:, 0:nsub, 2:3], in0=v0[0:npart, 0:nsub],
                scalar=1.0, in1=rr[0:npart, 0:nsub], op0=OP.mult, op1=OP.mult)

            # s = s~ * r ; p = exp(s) ; q = p * r
            s_sb = small.tile([128, 8, H], f32, tag="s")
            nc.vector.scalar_tensor_tensor(
                out=s_sb[0:npart, 0:nsub, :],
                in0=scps[0:npart, 0:nsub, 0:16], scalar=1.0,
                in1=rr[0:npart, 0:nsub, None].broadcast_to([npart, nsub, H]),
                op0=OP.mult, op1=OP.mult)
            qp_sb = small.tile([128, 8, 2 * H], bf16, tag="qp")
            if st["s"] == NSUPER:
                # tail super: zero all q/p rows, then fill the real tokens
                np_ = NREAL - N                          # rows < np_ are real
                nc.vector.memset(qp_sb[0:npart, 0:nsub, :], 0.0)
                nc.scalar.activation(qp_sb[0:np_, 0:nsub, 16:32],
                                     s_sb[0:np_, 0:nsub, :], AF.Exp)
                nc.vector.scalar_tensor_tensor(
                    out=qp_sb[0:np_, 0:nsub, 0:16],
                    in0=qp_sb[0:np_, 0:nsub, 16:32], scalar=1.0,
                    in1=rr[0:np_, 0:nsub, None].broadcast_to([np_, nsub, H]),
                    op0=OP.mult, op1=OP.mult)
            else:
                nc.scalar.activation(qp_sb[:, 0:nsub, 16:32],
                                     s_sb[:, 0:nsub, :], AF.Exp)
                nc.vector.scalar_tensor_tensor(
                    out=qp_sb[:, 0:nsub, 0:16],
                    in0=qp_sb[:, 0:nsub, 16:32], scalar=1.0,
                    in1=rr[:, 0:nsub, None].broadcast_to([128, nsub, H]),
                    op0=OP.mult, op1=OP.mult)
            st["qp"] = qp_sb

        def mix_super(st, mixps, denps, first_mm, last_s):
            """Accumulate mix/den matmuls for a chained super.

            One 16-col stationary (the q columns) serves all three matmuls:
            denps cols are [sum q | sum q*mu | sum q/r] = [junk | c1 | den]."""
            nsub, npart = st["nsub"], st["npart"]
            natT, aux, qp_sb = st["natT"], st["aux"], st["qp"]
            for j in range(nsub):
                last = last_s and (j == nsub - 1)
                nc.tensor.matmul(
                    mixps[:, 0:512], lhsT=qp_sb[0:npart, j, 0:16],
                    rhs=natT[0:npart, j, 0:512], start=first_mm, stop=last)
                nc.tensor.matmul(
                    mixps[:, 512:1024], lhsT=qp_sb[0:npart, j, 0:16],
                    rhs=natT[0:npart, j, 512:1024], start=first_mm, stop=last)
                nc.tensor.matmul(
                    denps[:, 0:3], lhsT=qp_sb[0:npart, j, 0:16],
                    rhs=aux[0:npart, j, 0:3], start=first_mm, stop=last)
                first_mm = False

        def epilogue_batch(b, mixps, denps):
            """Per-batch: head mix -> mixnT_all[:, :, :, b]."""
            dinv = ep_pool.tile([H, 1], f32, tag="dinv")
            nc.vector.reciprocal(dinv[:], denps[:, 2:3])
            c1 = ep_pool.tile([H, 1], f32, tag="c1")
            nc.vector.tensor_copy(c1[:], denps[:, 1:2])
            mixn = ep_pool.tile([H, D], bf16, tag="mixn")
            nc.vector.scalar_tensor_tensor(
                out=mixn[:], in0=mixps[:], scalar=c1[:],
                in1=dinv[:, 0:1].broadcast_to([H, D]),
                op0=OP.subtract, op1=OP.mult)
            tp_all = ps_tp.tile([128, 8, H], bf16, tag="tp")
            for c in range(8):
                nc.tensor.transpose(tp_all[:, c, :], mixn[:, 128 * c:128 * c + 128],
                                    ident_b[0:H, 0:H])
            nc.vector.tensor_copy(mixnT_all[:, :, :, b], tp_all[:])

        def tail_half(b0, qpools):
            """Per-core tail for batches [b0, b0+4): Wv/Wo matmuls, out-LN, head.

            qpools: (pool, tag) per psum tile in allocation order
            [ctxps, poolps, sums, bcast, ops_] — each entry must be a slot
            that is provably free during this tail (see call sites)."""
            BL = 4
            qi = [0]

            def qtile(shape, dt_, nm):
                pool, tg = qpools[qi[0]]
                qi[0] += 1
                return pool.tile(shape, dt_, tag=tg, name=f"tl{b0}_{nm}")

            ctxT_sb = singles.tile([128, 8, BLOC], bf16, tag="ctxT")
            poolT_sb = singles.tile([128, 8, BL], f32, tag="poolT")
            sq_sb = singles.tile([128, 8, BL], f32, tag="sq")
            # ctxT[o, b] = sum_d Wv'[o, d] mixn[head(o), d]  (block-diag heads)
            ctxps = qtile([128, 8, BL], f32, "ctxps")
            for k in range(8):
                for half in range(2):
                    h = 2 * k + half
                    for c in range(8):
                        nc.tensor.matmul(
                            ctxps[64 * half:64 * half + 64, k, :],
                            lhsT=wvT_sb[:, c, 64 * h:64 * h + 64],
                            rhs=mixnT_all[:, c, h, b0:b0 + BL],
                            start=(c == 0), stop=(c == 7))
            nc.vector.tensor_copy(ctxT_sb[:, :, b0:b0 + BL], ctxps[:])
            # pooledT[o2, b] = sum_o Wo[o2, o] ctx[o, b] + boT
            poolps = qtile([128, 8, BL], f32, "poolps")
            for k2 in range(8):
                for k in range(8):
                    nc.tensor.matmul(
                        poolps[:, k2, :],
                        lhsT=woT_sb[:, k, 128 * k2:128 * k2 + 128],
                        rhs=ctxT_sb[:, k, b0:b0 + BL],
                        start=(k == 0), stop=(k == 7))
            nc.vector.scalar_tensor_tensor(
                out=poolT_sb[:], in0=poolps[:], scalar=1.0,
                in1=bo_sb[:, :, None].broadcast_to([128, 8, BL]),
                op0=OP.mult, op1=OP.add)
            nc.scalar.square(sq_sb[:], poolT_sb[:])
            # LN stats over the o2 (partition+chunk) axis via ones-matmuls
            sums = qtile([1, 2 * BL], f32, "sums")
            for k2 in range(8):
                nc.tensor.matmul(sums[0:1, 0:BL], lhsT=onesf[:, 0:1],
                                 rhs=poolT_sb[:, k2, :],
                                 start=(k2 == 0), stop=(k2 == 7))
            for k2 in range(8):
                nc.tensor.matmul(sums[0:1, BL:2 * BL], lhsT=onesf[:, 0:1],
                                 rhs=sq_sb[:, k2, :],
                                 start=(k2 == 0), stop=(k2 == 7))
            stats = singles.tile([1, 2 * BL], f32, tag="stats")
            nc.vector.tensor_copy(stats[:], sums[:])
            v8 = singles.tile([1, BL], f32, tag="v8")
            nc.vector.scalar_tensor_tensor(
                out=v8[:], in0=stats[0:1, 0:BL],
                scalar=-1.0 / (1024.0 * 1024.0),
                in1=stats[0:1, 0:BL], op0=OP.mult, op1=OP.mult)
            nc.vector.scalar_tensor_tensor(
                out=v8[:], in0=stats[0:1, BL:2 * BL], scalar=1.0 / 1024.0,
                in1=v8[:], op0=OP.mult, op1=OP.add)
            nc.vector.tensor_scalar_add(v8[:], v8[:], EPS)
            r8 = singles.tile([1, BL], f32, tag="r8")
            nc.vector.reciprocal(r8[:], v8[:])
            nc.scalar.sqrt(r8[:], r8[:])                     # rsqrt(var+eps)
            pair = singles.tile([1, 2 * BL], f32, tag="pair")  # [-mu*r | r]
            nc.vector.scalar_tensor_tensor(
                out=pair[0:1, 0:BL], in0=stats[0:1, 0:BL], scalar=-1.0 / 1024.0,
                in1=r8[:], op0=OP.mult, op1=OP.mult)
            nc.vector.tensor_copy(pair[0:1, BL:2 * BL], r8[:])
            bcast = qtile([128, 2 * BL], f32, "bcast")
            nc.tensor.matmul(bcast[:], lhsT=ones_row[0:1, :], rhs=pair[:],
                             start=True, stop=True)
            nr_bc = singles.tile([128, 1, 2 * BL], f32, tag="nrbc")
            nc.vector.tensor_copy(nr_bc[:, 0, :], bcast[:])

            # yhatT = (poolT - mu) * r in [o2, b] layout, then the head matmul
            yhatT = singles.tile([128, 8, BL], bf16, tag="yhat")
            tn = singles.tile([128, 8, BL], f32, tag="tn")
            nc.vector.scalar_tensor_tensor(
                out=tn[:], in0=poolT_sb[:], scalar=1.0,
                in1=nr_bc[:, 0:1, BL:2 * BL].broadcast_to([128, 8, BL]),
                op0=OP.mult, op1=OP.mult)
            nc.vector.scalar_tensor_tensor(
                out=yhatT[:], in0=tn[:], scalar=1.0,
                in1=nr_bc[:, 0:1, 0:BL].broadcast_to([128, 8, BL]),
                op0=OP.mult, op1=OP.add)
            ops_ = qtile([C, BL], f32, "ops")
            for c in range(8):
                nc.tensor.matmul(ops_[:], lhsT=wpT_sb[:, c, :],
                                 rhs=yhatT[:, c, :],
                                 start=(c == 0), stop=(c == 7))
            out_sb = singles.tile([C, BL], f32, tag="outsb")
            nc.vector.tensor_scalar(out_sb[:], ops_[:], bp_sb[:], None,
                                    op0=OP.add)
            nc.sync.dma_start(out[b0:b0 + BL, :].rearrange("b c -> c b"),
                              out_sb[:])

        # ================= main software-pipelined loop ===================
        # stream of (b, s) supers; PE does scores(i) then mix(i-1)
        stream = [(b, s) for b in range(BLOC) for s in range(NSUPER + 1)]
        SPB = NSUPER + 1                     # supers per batch
        prev = None                          # state of super i-1 (loaded)
        mixq = {}                            # batch -> [mixps, denps, first_mm]

        for idx, (b, s) in enumerate(stream):
            if s == 0:
                mixq[b] = [ps_mix.tile([H, D], f32, tag="mix", name=f"mixps{b}"),
                           ps_den.tile([H, 3], f32, tag="den", name=f"denps{b}"),
                           True]
            st = load_super(b, s)            # DMA + x2 + scores for (b, s)
            if prev is not None:
                chain_super(prev)            # DVE/ACT chain for i-1
                pb = prev["b"]
                m = mixq[pb]
                mix_super(prev, m[0], m[1], m[2], last_s=(prev["s"] == NSUPER))
                m[2] = False
                if prev["s"] == NSUPER:      # batch pb fully accumulated
                    epilogue_batch(pb, m[0], m[1])
                    del mixq[pb]
                    if pb == 3:
                        # batch 3's mix slot was just released; batch 5's
                        # alloc comes later, so the tail may borrow it ONCE.
                        # Everything else funnels through the tp slot (the
                        # other mix slot and both scps slots are live, and
                        # borrowing them would deadlock DVE program order).
                        tail_half(0, [(ps_tp, "tp"), (ps_mix, "mix"),
                                      (ps_tp, "tp"), (ps_tp, "tp"),
                                      (ps_tp, "tp")])
            prev = st
            if idx == SPB + 2:
                # tail weights ride the (less loaded) SWDGE ring, late enough
                # not to starve the early nat loads
                nc.gpsimd.dma_start(wvT_sb[:], wvT[:])
                nc.gpsimd.dma_start(woT_sb[:], woT[:])
                nc.gpsimd.dma_start(wpT_sb[:], wpT[:])
                nc.gpsimd.dma_start(bo_sb[:], bo[:])
                nc.gpsimd.dma_start(bp_sb[:], bp[:])

        # drain the last super + last batch epilogue + second tail half
        chain_super(prev)
        m = mixq[prev["b"]]
        mix_super(prev, m[0], m[1], m[2], last_s=True)
        epilogue_batch(prev["b"], m[0], m[1])
        # the whole main loop is drained: every psum pool is free for the tail
        tail_half(4, [(ps_tp, "tp"), (ps_mix, "mix"), (ps_small, "scps"),
                      (ps_mix, "mix"), (ps_den, "den")])


def _build():
    import concourse.bass as bass
    import concourse.bacc as bacc
    import concourse.tile as tile
    from concourse import mybir

    f32 = mybir.dt.float32
    bf16 = mybir.dt.bfloat16

    nc = bacc.Bacc("TRN2", target_bir_lowering=False, debug=False,
                   num_devices=NCORES)
    io = {
        "toknatm": nc.dram_tensor("toknatm", [BLOC, NSUPER, 128, 8, D], bf16,
                                  kind="ExternalInput").ap(),
        "toknatt": nc.dram_tensor("toknatt", [BLOC, NTAIL, D], bf16,
                                  kind="ExternalInput").ap(),
        "tokTm": nc.dram_tensor("tokTm", [BLOC, NSUPER, 128, 8, SUPER],
                                mybir.dt.float8e4, kind="ExternalInput").ap(),
        "tokTt": nc.dram_tensor("tokTt", [BLOC, 128, 8, NTAIL],
                                mybir.dt.float8e4, kind="ExternalInput").ap(),
        "wsc": nc.dram_tensor("wsc", [128, 8, 17], bf16,
                              kind="ExternalInput").ap(),
        "wvT": nc.dram_tensor("wvT", [128, 8, D], bf16,
                              kind="ExternalInput").ap(),
        "woT": nc.dram_tensor("woT", [128, 8, D], bf16,
                              kind="ExternalInput").ap(),
        "wpT": nc.dram_tensor("wpT", [128, 8, C], bf16,
                              kind="ExternalInput").ap(),
        "bo": nc.dram_tensor("bo", [128, 8], f32, kind="ExternalInput").ap(),
        "bp": nc.dram_tensor("bp", [C, 1], f32, kind="ExternalInput").ap(),
        "out": nc.dram_tensor("out", [BLOC, C], f32,
                              kind="ExternalOutput").ap(),
    }
    with tile.TileContext(nc) as tc:
        _emit(tc, io)
    nc.compile()
    return nc


def _get_nc():
    if "nc" not in _cache:
        _cache["nc"] = _build()
    return _cache["nc"]


def run(inputs, trace=False, trace_kwargs=None):
    """Shard, run on 8 cores, gather.  Returns (out, BassKernelResults)."""
    from concourse.bass_utils import run_bass_kernel_spmd

    tokm, tokt, tokTm, tokTt, weights = _host_prep(inputs)
    nc = _get_nc()
    in_maps = []
    for i in range(NCORES):
        m = dict(weights)
        m["toknatm"] = np.ascontiguousarray(tokm[i * BLOC:(i + 1) * BLOC])
        m["toknatt"] = np.ascontiguousarray(tokt[i * BLOC:(i + 1) * BLOC])
        m["tokTm"] = np.ascontiguousarray(tokTm[i * BLOC:(i + 1) * BLOC])
        m["tokTt"] = np.ascontiguousarray(tokTt[i * BLOC:(i + 1) * BLOC])
        in_maps.append(m)
    res = run_bass_kernel_spmd(nc, in_maps, core_ids=list(range(NCORES)),
                               trace=trace, **(trace_kwargs or {}))
    out = np.concatenate([np.asarray(res.results[i]["out"], dtype=np.float32)
                          for i in range(NCORES)], axis=0)
    return out, res


def kernel(**inputs):
    out, _ = run(inputs)
    return out


# revision 40
# speedup vs baseline: 1.0937x; 1.0937x over previous
"""AttentionPoolHead Trainium2 kernel (8 NeuronCores, batch-data-parallel).

Takes FULL inputs (as produced by setup_inputs), returns FULL (B, C) output.

Math (exact rewrite of the reference):
  tokens = [patches | cls | storage | zero-pad]            (order irrelevant: softmax-pool
                                                            is permutation invariant)
  kv     = LN(tokens) ; q fixed vector  =>  per-token score collapses to
      s[t,h] = r_t * (x_t . w''[:,h])
  with w'' = (Wk_head.T @ qp) * g / sqrt(HD), mean-centered over D (the -mu term of LN
  and all per-head constants vanish under softmax).  With q_t = p_t * r_t,
  p_t = exp(s_t):
      head_mix_h = ((sum_t q_t x_t) - rowmean correction) / (sum_t p_t)
      ctx = Wv' @ head_mix (+ folded biases), pooled = Wo @ ctx + bo', LN, Wp'' head.

Pipeline notes (v2):
  - token stream is read twice: bf16 natural layout (mix matmul rhs) and f8
    transposed per-super-contiguous layout (score matmul stationary).
  - per-token sum(x^2) is split across ACT (4/8), DVE (2/8), GPSIMD (2/8).
  - PE emission is software-pipelined one super: scores(s) ; mix(s-1), so the
    DVE score->qp chain of super s-1 overlaps the score matmuls of super s.
  - the per-core tail runs twice on batch halves (0-3 after batch 3, 4-7 at
    the end) so most of it hides under the second half's token streaming.
"""

import numpy as np

B, S, N, D, H, C = 64, 4, 4096, 1024, 16, 14
HD = D // H
EPS = 1e-5
NCORES = 8
BLOC = B // NCORES          # batches per core
NREAL = 1 + S + N           # 4101 real tokens
NTAIL = 8                   # tail mini-super: 5 real + 3 zero-pad tokens
LTOT = N + NTAIL            # 4104 tokens kept per batch
NSUPER = 4                  # full supers of 1024 tokens (the patches region)
SUPER = 1024
NSUB = 8                    # 128-token subtiles per full super

_cache = {}


def _f32(x):
    return np.ascontiguousarray(np.asarray(x, dtype=np.float32))


def _host_prep(inputs):
    """All weight folding + token layout prep on the host (numpy)."""
    import ml_dtypes

    bf16 = ml_dtypes.bfloat16
    f8 = ml_dtypes.float8_e4m3

    cls_tok = _f32(inputs["cls_tok"])        # [B, D]
    storage = _f32(inputs["storage"])        # [B, S, D]
    patches = _f32(inputs["patches"])        # [B, N, D]
    query = _f32(inputs["query"]).reshape(D)
    g_kv = _f32(inputs["ln_kv_g"])
    b_kv = _f32(inputs["ln_kv_b"])
    Wq = _f32(inputs["Wq"]); Wk = _f32(inputs["Wk"]); Wv = _f32(inputs["Wv"])
    bq = _f32(inputs["bq"])
    Wo = _f32(inputs["Wo"]); bo = _f32(inputs["bo"])
    g_out = _f32(inputs["ln_out_g"]); b_out = _f32(inputs["ln_out_b"])
    Wp = _f32(inputs["Wp"]); bp = _f32(inputs["bp"])

    # --- score weights: s[t,h] = r_t * (x_t . w''[:, h]) ----------------------
    qp = query @ Wq.T + bq                                   # [D]
    w_raw = np.einsum("hid,hi->dh", Wk.reshape(H, HD, D), qp.reshape(H, HD))
    w_raw /= np.sqrt(HD).astype(np.float32)
    wpr = w_raw * g_kv[:, None]                              # fold LN gain
    wpp = wpr - wpr.mean(0, keepdims=True)                   # fold LN mean-centering
    # device layout [128, 8, 17]: [:, c, 0:16] = wpp[128c+p, :], col 16 = ones
    wsc_dev = np.zeros((128, 8, 17), dtype=np.float32)
    wsc_dev[:, :, :16] = wpp.reshape(8, 128, 16).transpose(1, 0, 2)
    wsc_dev[:, :, 16] = 1.0
    wsc_dev = wsc_dev.astype(bf16)

    # --- Wv / Wo / Wp folds ---------------------------------------------------
    WvT = (Wv * g_kv[None, :]).T                             # [D_in, D_out]
    wvT_dev = np.ascontiguousarray(
        WvT.reshape(8, 128, D).transpose(1, 0, 2)).astype(bf16)   # [128, 8, 1024]
    woT_dev = np.ascontiguousarray(
        Wo.T.reshape(8, 128, D).transpose(1, 0, 2)).astype(bf16)  # [128, 8, 1024]
    WpT = (Wp * g_out[None, :]).T                            # [D, C]
    wpT_dev = np.ascontiguousarray(
        WpT.reshape(8, 128, C).transpose(1, 0, 2)).astype(bf16)   # [128, 8, 14]

    bo_comb = bo + Wo @ (Wv @ b_kv)                          # [D]
    bo_dev = np.ascontiguousarray(
        bo_comb.reshape(8, 128).T).astype(np.float32)        # [128, 8] = boT
    bp_comb = (bp + Wp @ b_out).reshape(C, 1).astype(np.float32)  # [14, 1]

    # --- token stream in both layouts ----------------------------------------
    # natural: order patches (0..N-1), cls, storage, pad
    tok = np.zeros((B, LTOT, D), dtype=bf16)
    tok[:, :N] = patches.astype(bf16)
    tok[:, N] = cls_tok.astype(bf16)
    tok[:, N + 1:N + 1 + S] = storage.astype(bf16)
    # natural layout stays token-major: the DMA rearrange makes 2 KB
    # per-partition descriptors, which interleave across partitions (16 KB
    # p-major bursts measurably stall engine SBUF reads)
    # transposed f8, p-major per-super contiguous:
    #   tokT_main[b, s, p, c, t] = tok[b, 1024 s + t, 128 c + p]
    tokf8 = tok[:, :N].astype(f8)                            # [B, N, D]
    tokT_main = np.ascontiguousarray(
        tokf8.reshape(B, NSUPER, SUPER, 8, 128).transpose(0, 1, 4, 3, 2))
    #   tokT_tail[b, p, c, t] = tok[b, N + t, 128 c + p]
    tokT_tail = np.ascontiguousarray(
        tok[:, N:].astype(f8).reshape(B, NTAIL, 8, 128).transpose(0, 3, 2, 1))

    weights = dict(wsc=wsc_dev, wvT=wvT_dev, woT=woT_dev, wpT=wpT_dev,
                   bo=bo_dev, bp=bp_comb)
    return tok, tokT_main, tokT_tail, weights


def _emit(tc, io):
    """Emit the Tile program for one core (BLOC batches)."""
    import concourse.bass as bass
    from concourse import mybir

    nc = tc.nc
    f32 = mybir.dt.float32
    bf16 = mybir.dt.bfloat16
    f8 = mybir.dt.float8e4
    AF = mybir.ActivationFunctionType
    OP = mybir.AluOpType

    toknat, tokTm, tokTt, wsc, wvT, woT, wpT, bo, bp, out = (
        io["toknat"], io["tokTm"], io["tokTt"], io["wsc"],
        io["wvT"], io["woT"], io["wpT"], io["bo"], io["bp"], io["out"])

    from contextlib import ExitStack
    ctx = ExitStack()
    with ctx:
        singles = ctx.enter_context(tc.tile_pool(name="singles", bufs=1))
        nat_pool = ctx.enter_context(tc.tile_pool(name="nat", bufs=5))
        tt_pool = ctx.enter_context(tc.tile_pool(name="tt", bufs=5))
        tl_pool = ctx.enter_context(tc.tile_pool(name="tl", bufs=2))
        small = ctx.enter_context(tc.tile_pool(name="small", bufs=3))
        ep_pool = ctx.enter_context(tc.tile_pool(name="ep", bufs=2))
        ps_small = ctx.enter_context(tc.tile_pool(name="ps_small", bufs=2, space="PSUM"))
        ps_mix = ctx.enter_context(tc.tile_pool(name="ps_mix", bufs=2, space="PSUM"))
        ps_den = ctx.enter_context(tc.tile_pool(name="ps_den", bufs=1, space="PSUM"))
        ps_tp = ctx.enter_context(tc.tile_pool(name="ps_tp", bufs=1, space="PSUM"))

        # ---- score weights (needed from the first super) ----------------
        wsc_sb = singles.tile([128, 8, 17], bf16)
        nc.sync.dma_start(wsc_sb[:], wsc[:])
        # tail weights are DMA'd once batch 0's token loads are queued
        wvT_sb = singles.tile([128, 8, D], bf16)
        woT_sb = singles.tile([128, 8, D], bf16)
        wpT_sb = singles.tile([128, 8, C], bf16)
        bo_sb = singles.tile([128, 8], f32)      # boT[p, k2] = bo_comb[128*k2+p]
        bp_sb = singles.tile([C, 1], f32)

        from concourse.masks import make_identity
        ident_b = singles.tile([128, 128], bf16)
        make_identity(nc, ident_b[:])
        onesf = singles.tile([128, 1], f32)
        nc.vector.memset(onesf[:], 1.0)
        ones_row = singles.tile([1, 128], f32)
        nc.vector.memset(ones_row[:], 1.0)

        # persistent per-core accumulators
        mixnT_all = singles.tile([128, 8, H, BLOC], bf16)    # [dp, c, h, b]

        x2junk = singles.tile([128, 1024], bf16)
        x2junk_a = singles.tile([128, 1024], bf16)

        def load_super(b, s):
            """Issue DMAs + x^2 + scores for (b, s); return state dict."""
            if s < NSUPER:
                natT = nat_pool.tile([128, NSUB, 1024], bf16, tag="nat")
                aux = nat_pool.tile([128, NSUB, 3], bf16, tag="aux")
                ttT = tt_pool.tile([128, 8, SUPER], f8, tag="tt")
                t0 = SUPER * s
                # split the nat load 6/2 across the HWDGE and SWDGE rings so
                # both carry ~1.5 MB/super (SDMA round-robins between rings)
                nc.sync.dma_start(
                    natT[:, 0:6, :],
                    toknat[b, t0:t0 + 768, :]
                    .rearrange("(j p) d -> p j d", p=128))
                nc.gpsimd.dma_start(ttT[:], tokTm[b, s])
                nc.gpsimd.dma_start(
                    natT[:, 6:8, :],
                    toknat[b, t0 + 768:t0 + 1024, :]
                    .rearrange("(j p) d -> p j d", p=128))
                nc.vector.memset(aux[:, :, 0:1], 1.0)
                nsub, npart = NSUB, 128
            else:
                natT = tl_pool.tile([NTAIL, 1, 1024], bf16, tag="natl")
                aux = tl_pool.tile([NTAIL, 1, 3], bf16, tag="auxl")
                ttT = tl_pool.tile([128, 8, NTAIL], f8, tag="ttl")
                nc.gpsimd.dma_start(ttT[:], tokTt[b])
                nc.sync.dma_start(
                    natT[:],
                    toknat[b, N:N + NTAIL, :].rearrange("(j p) d -> p j d", p=NTAIL))
                nc.vector.memset(aux[:, :, 0:1], 1.0)
                nsub, npart = 1, NTAIL

            # per-token sum of squares, split DVE 3/8 + ACT 5/8
            x2acc = small.tile([128, 8], f32, tag="x2acc")
            for j in range(nsub):
                if s == NSUPER:
                    src = natT[:, j, 0:1024]     # [8, 1024]
                    nc.vector.scalar_tensor_tensor(
                        out=x2junk[0:npart, :], in0=src, scalar=1.0, in1=src,
                        op0=OP.mult, op1=OP.mult,
                        accum_out=x2acc[0:npart, j:j + 1])
                elif j in (0, 3, 6):
                    nc.vector.scalar_tensor_tensor(
                        out=x2junk[:], in0=natT[:, j, 0:1024], scalar=1.0,
                        in1=natT[:, j, 0:1024], op0=OP.mult, op1=OP.mult,
                        accum_out=x2acc[:, j:j + 1])
                else:
                    nc.scalar.activation(
                        x2junk_a[:], natT[:, j, 0:1024], AF.Square,
                        accum_out=x2acc[:, j:j + 1])

            # scores: s~[tok, h] (+ col16 = sum_d x) accumulated over 8 D-chunks
            scps = ps_small.tile([128, 8, 17], f32, tag="scps")
            ntok = 128 if s < NSUPER else NTAIL
            for j in range(nsub):
                for c in range(8):
                    nc.tensor.matmul(
                        scps[0:npart, j, :],
                        lhsT=ttT[:, c, 128 * j:128 * j + ntok],
                        rhs=wsc_sb[:, c, :],
                        start=(c == 0), stop=(c == 7))
            return dict(natT=natT, aux=aux, ttT=ttT, x2acc=x2acc, scps=scps,
                        nsub=nsub, npart=npart, s=s, b=b)

        def chain_super(st):
            """LN stats -> r -> softmax weights for a loaded super (DVE/ACT)."""
            nsub, npart = st["nsub"], st["npart"]
            natT, aux, x2acc, scps = (st["natT"], st["aux"], st["x2acc"],
                                      st["scps"])
            sx = small.tile([128, 8], f32, tag="sx")
            nc.vector.tensor_copy(sx[0:npart, 0:nsub], scps[0:npart, 0:nsub, 16])
            # stash mu_t = sum(x)/1024 into aux column 1 (den matmul rhs)
            nc.vector.tensor_scalar_mul(aux[:, 0:nsub, 1:2],
                                        sx[0:npart, 0:nsub], 1.0 / 1024.0)
            v0 = small.tile([128, 8], f32, tag="v0")
            nc.vector.scalar_tensor_tensor(
                out=v0[0:npart, 0:nsub],
                in0=sx[0:npart, 0:nsub], scalar=-1.0 / (1024.0 * 1024.0),
                in1=sx[0:npart, 0:nsub], op0=OP.mult, op1=OP.mult)
            nc.vector.scalar_tensor_tensor(
                out=v0[0:npart, 0:nsub],
                in0=x2acc[0:npart, 0:nsub], scalar=1.0 / 1024.0,
                in1=v0[0:npart, 0:nsub], op0=OP.mult, op1=OP.add)
            nc.vector.tensor_scalar_add(v0[0:npart, 0:nsub],
                                        v0[0:npart, 0:nsub], EPS)
            # r = rsqrt(wn) via 2 Newton steps from seed 1.0 (wn ~= 1 +- 0.07)
            rr = small.tile([128, 8], f32, tag="rr")
            ra = small.tile([128, 8], f32, tag="ra")
            rc = small.tile([128, 8], f32, tag="rc")
            nc.vector.tensor_scalar(rr[0:npart, 0:nsub], v0[0:npart, 0:nsub],
                                    -0.5, 1.5, op0=OP.mult, op1=OP.add)
            nc.vector.scalar_tensor_tensor(
                out=ra[0:npart, 0:nsub], in0=rr[0:npart, 0:nsub], scalar=1.0,
                in1=rr[0:npart, 0:nsub], op0=OP.mult, op1=OP.mult)
            nc.vector.scalar_tensor_tensor(
                out=rc[0:npart, 0:nsub], in0=v0[0:npart, 0:nsub], scalar=-0.5,
                in1=ra[0:npart, 0:nsub], op0=OP.mult, op1=OP.mult)
            nc.vector.scalar_tensor_tensor(
                out=rr[0:npart, 0:nsub], in0=rc[0:npart, 0:nsub], scalar=1.5,
                in1=rr[0:npart, 0:nsub], op0=OP.add, op1=OP.mult)
            # stash 1/r = wn * r into aux column 2: the den matmul then
            # accumulates sum_t q/r = sum_t p from the q-columns alone
            nc.vector.scalar_tensor_tensor(
                out=aux[:, 0:nsub, 2:3], in0=v0[0:npart, 0:nsub],
                scalar=1.0, in1=rr[0:npart, 0:nsub], op0=OP.mult, op1=OP.mult)

            # s = s~ * r ; p = exp(s) ; q = p * r
            s_sb = small.tile([128, 8, H], f32, tag="s")
            nc.vector.scalar_tensor_tensor(
                out=s_sb[0:npart, 0:nsub, :],
                in0=scps[0:npart, 0:nsub, 0:16], scalar=1.0,
                in1=rr[0:npart, 0:nsub, None].broadcast_to([npart, nsub, H]),
                op0=OP.mult, op1=OP.mult)
            qp_sb = small.tile([128, 8, 2 * H], bf16, tag="qp")
            if st["s"] == NSUPER:
                # tail super: zero all q/p rows, then fill the real tokens
                np_ = NREAL - N                          # rows < np_ are real
                nc.vector.memset(qp_sb[0:npart, 0:nsub, :], 0.0)
                nc.scalar.activation(qp_sb[0:np_, 0:nsub, 16:32],
                                     s_sb[0:np_, 0:nsub, :], AF.Exp)
                nc.vector.scalar_tensor_tensor(
                    out=qp_sb[0:np_, 0:nsub, 0:16],
                    in0=qp_sb[0:np_, 0:nsub, 16:32], scalar=1.0,
                    in1=rr[0:np_, 0:nsub, None].broadcast_to([np_, nsub, H]),
                    op0=OP.mult, op1=OP.mult)
            else:
                nc.scalar.activation(qp_sb[:, 0:nsub, 16:32],
                                     s_sb[:, 0:nsub, :], AF.Exp)
                nc.vector.scalar_tensor_tensor(
                    out=qp_sb[:, 0:nsub, 0:16],
                    in0=qp_sb[:, 0:nsub, 16:32], scalar=1.0,
                    in1=rr[:, 0:nsub, None].broadcast_to([128, nsub, H]),
                    op0=OP.mult, op1=OP.mult)
            st["qp"] = qp_sb

        def mix_super(st, mixps, denps, first_mm, last_s):
            """Accumulate mix/den matmuls for a chained super.

            One 16-col stationary (the q columns) serves all three matmuls:
            denps cols are [sum q | sum q*mu | sum q/r] = [junk | c1 | den]."""
            nsub, npart = st["nsub"], st["npart"]
            natT, aux, qp_sb = st["natT"], st["aux"], st["qp"]
            for j in range(nsub):
                last = last_s and (j == nsub - 1)
                nc.tensor.matmul(
                    mixps[:, 0:512], lhsT=qp_sb[0:npart, j, 0:16],
                    rhs=natT[0:npart, j, 0:512], start=first_mm, stop=last)
                nc.tensor.matmul(
                    mixps[:, 512:1024], lhsT=qp_sb[0:npart, j, 0:16],
                    rhs=natT[0:npart, j, 512:1024], start=first_mm, stop=last)
                nc.tensor.matmul(
                    denps[:, 0:3], lhsT=qp_sb[0:npart, j, 0:16],
                    rhs=aux[0:npart, j, 0:3], start=first_mm, stop=last)
                first_mm = False

        def epilogue_batch(b, mixps, denps):
            """Per-batch: head mix -> mixnT_all[:, :, :, b]."""
            dinv = ep_pool.tile([H, 1], f32, tag="dinv")
            nc.vector.reciprocal(dinv[:], denps[:, 2:3])
            c1 = ep_pool.tile([H, 1], f32, tag="c1")
            nc.vector.tensor_copy(c1[:], denps[:, 1:2])
            mixn = ep_pool.tile([H, D], bf16, tag="mixn")
            nc.vector.scalar_tensor_tensor(
                out=mixn[:], in0=mixps[:], scalar=c1[:],
                in1=dinv[:, 0:1].broadcast_to([H, D]),
                op0=OP.subtract, op1=OP.mult)
            tp_all = ps_tp.tile([128, 8, H], bf16, tag="tp")
            for c in range(8):
                nc.tensor.transpose(tp_all[:, c, :], mixn[:, 128 * c:128 * c + 128],
                                    ident_b[0:H, 0:H])
            nc.vector.tensor_copy(mixnT_all[:, :, :, b], tp_all[:])

        def tail_half(b0, qpools):
            """Per-core tail for batches [b0, b0+4): Wv/Wo matmuls, out-LN, head.

            qpools: (pool, tag) per psum tile in allocation order
            [ctxps, poolps, sums, bcast, ops_] — each entry must be a slot
            that is provably free during this tail (see call sites)."""
            BL = 4
            qi = [0]

            def qtile(shape, dt_, nm):
                pool, tg = qpools[qi[0]]
                qi[0] += 1
                return pool.tile(shape, dt_, tag=tg, name=f"tl{b0}_{nm}")

            ctxT_sb = singles.tile([128, 8, BLOC], bf16, tag="ctxT")
            poolT_sb = singles.tile([128, 8, BL], f32, tag="poolT")
            sq_sb = singles.tile([128, 8, BL], f32, tag="sq")
            # ctxT[o, b] = sum_d Wv'[o, d] mixn[head(o), d]  (block-diag heads)
            ctxps = qtile([128, 8, BL], f32, "ctxps")
            for k in range(8):
                for half in range(2):
                    h = 2 * k + half
                    for c in range(8):
                        nc.tensor.matmul(
                            ctxps[64 * half:64 * half + 64, k, :],
                            lhsT=wvT_sb[:, c, 64 * h:64 * h + 64],
                            rhs=mixnT_all[:, c, h, b0:b0 + BL],
                            start=(c == 0), stop=(c == 7))
            nc.vector.tensor_copy(ctxT_sb[:, :, b0:b0 + BL], ctxps[:])
            # pooledT[o2, b] = sum_o Wo[o2, o] ctx[o, b] + boT
            poolps = qtile([128, 8, BL], f32, "poolps")
            for k2 in range(8):
                for k in range(8):
                    nc.tensor.matmul(
                        poolps[:, k2, :],
                        lhsT=woT_sb[:, k, 128 * k2:128 * k2 + 128],
                        rhs=ctxT_sb[:, k, b0:b0 + BL],
                        start=(k == 0), stop=(k == 7))
            nc.vector.scalar_tensor_tensor(
                out=poolT_sb[:], in0=poolps[:], scalar=1.0,
                in1=bo_sb[:, :, None].broadcast_to([128, 8, BL]),
                op0=OP.mult, op1=OP.add)
            nc.scalar.square(sq_sb[:], poolT_sb[:])
            # LN stats over the o2 (partition+chunk) axis via ones-matmuls
            sums = qtile([1, 2 * BL], f32, "sums")
            for k2 in range(8):
                nc.tensor.matmul(sums[0:1, 0:BL], lhsT=onesf[:, 0:1],
                                 rhs=poolT_sb[:, k2, :],
                                 start=(k2 == 0), stop=(k2 == 7))
            for k2 in range(8):
                nc.tensor.matmul(sums[0:1, BL:2 * BL], lhsT=onesf[:, 0:1],
                                 rhs=sq_sb[:, k2, :],
                                 start=(k2 == 0), stop=(k2 == 7))
            stats = singles.tile([1, 2 * BL], f32, tag="stats")
            nc.vector.tensor_copy(stats[:], sums[:])
            v8 = singles.tile([1, BL], f32, tag="v8")
            nc.vector.scalar_tensor_tensor(
                out=v8[:], in0=stats[0:1, 0:BL],
                scalar=-1.0 / (1024.0 * 1024.0),
                in1=stats[0:1, 0:BL], op0=OP.mult, op1=OP.mult)
            nc.vector.scalar_tensor_tensor(
                out=v8[:], in0=stats[0:1, BL:2 * BL], scalar=1.0 / 1024.0,
                in1=v8[:], op0=OP.mult, op1=OP.add)
            nc.vector.tensor_scalar_add(v8[:], v8[:], EPS)
            r8 = singles.tile([1, BL], f32, tag="r8")
            nc.vector.reciprocal(r8[:], v8[:])
            nc.scalar.sqrt(r8[:], r8[:])                     # rsqrt(var+eps)
            pair = singles.tile([1, 2 * BL], f32, tag="pair")  # [-mu*r | r]
            nc.vector.scalar_tensor_tensor(
                out=pair[0:1, 0:BL], in0=stats[0:1, 0:BL], scalar=-1.0 / 1024.0,
                in1=r8[:], op0=OP.mult, op1=OP.mult)
            nc.vector.tensor_copy(pair[0:1, BL:2 * BL], r8[:])
            bcast = qtile([128, 2 * BL], f32, "bcast")
            nc.tensor.matmul(bcast[:], lhsT=ones_row[0:1, :], rhs=pair[:],
                             start=True, stop=True)
            nr_bc = singles.tile([128, 1, 2 * BL], f32, tag="nrbc")
            nc.vector.tensor_copy(nr_bc[:, 0, :], bcast[:])

            # yhatT = (poolT - mu) * r in [o2, b] layout, then the head matmul
            yhatT = singles.tile([128, 8, BL], bf16, tag="yhat")
            tn = singles.tile([128, 8, BL], f32, tag="tn")
            nc.vector.scalar_tensor_tensor(
                out=tn[:], in0=poolT_sb[:], scalar=1.0,
                in1=nr_bc[:, 0:1, BL:2 * BL].broadcast_to([128, 8, BL]),
                op0=OP.mult, op1=OP.mult)
            nc.vector.scalar_tensor_tensor(
                out=yhatT[:], in0=tn[:], scalar=1.0,
                in1=nr_bc[:, 0:1, 0:BL].broadcast_to([128, 8, BL]),
                op0=OP.mult, op1=OP.add)
            ops_ = qtile([C, BL], f32, "ops")
            for c in range(8):
                nc.tensor.matmul(ops_[:], lhsT=wpT_sb[:, c, :],
                                 rhs=yhatT[:, c, :],
                                 start=(c == 0), stop=(c == 7))
            out_sb = singles.tile([C, BL], f32, tag="outsb")
            nc.vector.tensor_scalar(out_sb[:], ops_[:], bp_sb[:], None,
                                    op0=OP.add)
            nc.sync.dma_start(out[b0:b0 + BL, :].rearrange("b c -> c b"),
                              out_sb[:])

        # ================= main software-pipelined loop ===================
        # stream of (b, s) supers; PE does scores(i) then mix(i-1)
        stream = [(b, s) for b in range(BLOC) for s in range(NSUPER + 1)]
        SPB = NSUPER + 1                     # supers per batch
        prev = None                          # state of super i-1 (loaded)
        mixq = {}                            # batch -> [mixps, denps, first_mm]

        for idx, (b, s) in enumerate(stream):
            if s == 0:
                mixq[b] = [ps_mix.tile([H, D], f32, tag="mix", name=f"mixps{b}"),
                           ps_den.tile([H, 3], f32, tag="den", name=f"denps{b}"),
                           True]
            st = load_super(b, s)            # DMA + x2 + scores for (b, s)
            if prev is not None:
                chain_super(prev)            # DVE/ACT chain for i-1
                pb = prev["b"]
                m = mixq[pb]
                mix_super(prev, m[0], m[1], m[2], last_s=(prev["s"] == NSUPER))
                m[2] = False
                if prev["s"] == NSUPER:      # batch pb fully accumulated
                    epilogue_batch(pb, m[0], m[1])
                    del mixq[pb]
                    if pb == 3:
                        # batch 3's mix slot was just released; batch 5's
                        # alloc comes later, so the tail may borrow it ONCE.
                        # Everything else funnels through the tp slot (the
                        # other mix slot and both scps slots are live, and
                        # borrowing them would deadlock DVE program order).
                        tail_half(0, [(ps_tp, "tp"), (ps_mix, "mix"),
                                      (ps_tp, "tp"), (ps_tp, "tp"),
                                      (ps_tp, "tp")])
            prev = st
            if idx == SPB + 2:
                # tail weights ride the (less loaded) SWDGE ring, late enough
                # not to starve the early nat loads
                nc.gpsimd.dma_start(wvT_sb[:], wvT[:])
                nc.gpsimd.dma_start(woT_sb[:], woT[:])
                nc.gpsimd.dma_start(wpT_sb[:], wpT[:])
                nc.gpsimd.dma_start(bo_sb[:], bo[:])
                nc.gpsimd.dma_start(bp_sb[:], bp[:])

        # drain the last super + last batch epilogue + second tail half
        chain_super(prev)
        m = mixq[prev["b"]]
        mix_super(prev, m[0], m[1], m[2], last_s=True)
        epilogue_batch(prev["b"], m[0], m[1])
        # the whole main loop is drained: every psum pool is free for the tail
        tail_half(4, [(ps_tp, "tp"), (ps_mix, "mix"), (ps_small, "scps"),
                      (ps_mix, "mix"), (ps_den, "den")])


def _build():
    import concourse.bass as bass
    import concourse.bacc as bacc
    import concourse.tile as tile
    from concourse import mybir

    f32 = mybir.dt.float32
    bf16 = mybir.dt.bfloat16

    nc = bacc.Bacc("TRN2", target_bir_lowering=False, debug=False,
                   num_devices=NCORES)
    io = {
        "toknat": nc.dram_tensor("toknat", [BLOC, LTOT, D], bf16,
                                 kind="ExternalInput").ap(),
        "tokTm": nc.dram_tensor("tokTm", [BLOC, NSUPER, 128, 8, SUPER],
                                mybir.dt.float8e4, kind="ExternalInput").ap(),
        "tokTt": nc.dram_tensor("tokTt", [BLOC, 128, 8, NTAIL],
                                mybir.dt.float8e4, kind="ExternalInput").ap(),
        "wsc": nc.dram_tensor("wsc", [128, 8, 17], bf16,
                              kind="ExternalInput").ap(),
        "wvT": nc.dram_tensor("wvT", [128, 8, D], bf16,
                              kind="ExternalInput").ap(),
        "woT": nc.dram_tensor("woT", [128, 8, D], bf16,
                              kind="ExternalInput").ap(),
        "wpT": nc.dram_tensor("wpT", [128, 8, C], bf16,
                              kind="ExternalInput").ap(),
        "bo": nc.dram_tensor("bo", [128, 8], f32, kind="ExternalInput").ap(),
        "bp": nc.dram_tensor("bp", [C, 1], f32, kind="ExternalInput").ap(),
        "out": nc.dram_tensor("out", [BLOC, C], f32,
                              kind="ExternalOutput").ap(),
    }
    with tile.TileContext(nc) as tc:
        _emit(tc, io)
    nc.compile()
    return nc


def _get_nc():
    if "nc" not in _cache:
        _cache["nc"] = _build()
    return _cache["nc"]


def run(inputs, trace=False, trace_kwargs=None):
    """Shard, run on 8 cores, gather.  Returns (out, BassKernelResults)."""
    from concourse.bass_utils import run_bass_kernel_spmd

    tok, tokTm, tokTt, weights = _host_prep(inputs)
    nc = _get_nc()
    in_maps = []
    for i in range(NCORES):
        m = dict(weights)
        m["toknat"] = np.ascontiguousarray(tok[i * BLOC:(i + 1) * BLOC])
        m["tokTm"] = np.ascontiguousarray(tokTm[i * BLOC:(i + 1) * BLOC])
        m["tokTt"] = np.ascontiguousarray(tokTt[i * BLOC:(i + 1) * BLOC])
        in_maps.append(m)
    res = run_bass_kernel_spmd(nc, in_maps, core_ids=list(range(NCORES)),
                               trace=trace, **(trace_kwargs or {}))
    out = np.concatenate([np.asarray(res.results[i]["out"], dtype=np.float32)
                          for i in range(NCORES)], axis=0)
    return out, res


def kernel(**inputs):
    out, _ = run(inputs)
    return out


# revision 43
# speedup vs baseline: 1.1124x; 1.0171x over previous
"""AttentionPoolHead Trainium2 kernel (8 NeuronCores, batch-data-parallel).

Takes FULL inputs (as produced by setup_inputs), returns FULL (B, C) output.

Math (exact rewrite of the reference):
  tokens = [patches | cls | storage | zero-pad]            (order irrelevant: softmax-pool
                                                            is permutation invariant)
  kv     = LN(tokens) ; q fixed vector  =>  per-token score collapses to
      s[t,h] = r_t * (x_t . w''[:,h])
  with w'' = (Wk_head.T @ qp) * g / sqrt(HD), mean-centered over D (the -mu term of LN
  and all per-head constants vanish under softmax).  With q_t = p_t * r_t,
  p_t = exp(s_t):
      head_mix_h = ((sum_t q_t x_t) - rowmean correction) / (sum_t p_t)
      ctx = Wv' @ head_mix (+ folded biases), pooled = Wo @ ctx + bo', LN, Wp'' head.

Pipeline notes (v2):
  - token stream is read twice: bf16 natural layout (mix matmul rhs) and f8
    transposed per-super-contiguous layout (score matmul stationary).
  - per-token sum(x^2) is split across ACT (4/8), DVE (2/8), GPSIMD (2/8).
  - PE emission is software-pipelined one super: scores(s) ; mix(s-1), so the
    DVE score->qp chain of super s-1 overlaps the score matmuls of super s.
  - the per-core tail runs twice on batch halves (0-3 after batch 3, 4-7 at
    the end) so most of it hides under the second half's token streaming.
"""

import numpy as np

B, S, N, D, H, C = 64, 4, 4096, 1024, 16, 14
HD = D // H
EPS = 1e-5
NCORES = 8
BLOC = B // NCORES          # batches per core
NREAL = 1 + S + N           # 4101 real tokens
NTAIL = 8                   # tail mini-super: 5 real + 3 zero-pad tokens
LTOT = N + NTAIL            # 4104 tokens kept per batch
NSUPER = 4                  # full supers of 1024 tokens (the patches region)
SUPER = 1024
NSUB = 8                    # 128-token subtiles per full super

_cache = {}


def _f32(x):
    return np.ascontiguousarray(np.asarray(x, dtype=np.float32))


def _host_prep(inputs):
    """All weight folding + token layout prep on the host (numpy)."""
    import ml_dtypes

    bf16 = ml_dtypes.bfloat16
    f8 = ml_dtypes.float8_e4m3

    cls_tok = _f32(inputs["cls_tok"])        # [B, D]
    storage = _f32(inputs["storage"])        # [B, S, D]
    patches = _f32(inputs["patches"])        # [B, N, D]
    query = _f32(inputs["query"]).reshape(D)
    g_kv = _f32(inputs["ln_kv_g"])
    b_kv = _f32(inputs["ln_kv_b"])
    Wq = _f32(inputs["Wq"]); Wk = _f32(inputs["Wk"]); Wv = _f32(inputs["Wv"])
    bq = _f32(inputs["bq"])
    Wo = _f32(inputs["Wo"]); bo = _f32(inputs["bo"])
    g_out = _f32(inputs["ln_out_g"]); b_out = _f32(inputs["ln_out_b"])
    Wp = _f32(inputs["Wp"]); bp = _f32(inputs["bp"])

    # --- score weights: s[t,h] = r_t * (x_t . w''[:, h]) ----------------------
    qp = query @ Wq.T + bq                                   # [D]
    w_raw = np.einsum("hid,hi->dh", Wk.reshape(H, HD, D), qp.reshape(H, HD))
    w_raw /= np.sqrt(HD).astype(np.float32)
    wpr = w_raw * g_kv[:, None]                              # fold LN gain
    wpp = wpr - wpr.mean(0, keepdims=True)                   # fold LN mean-centering
    # device layout [128, 8, 17]: [:, c, 0:16] = wpp[128c+p, :], col 16 = ones
    wsc_dev = np.zeros((128, 8, 17), dtype=np.float32)
    wsc_dev[:, :, :16] = wpp.reshape(8, 128, 16).transpose(1, 0, 2)
    wsc_dev[:, :, 16] = 1.0
    wsc_dev = wsc_dev.astype(bf16)

    # --- Wv / Wo / Wp folds ---------------------------------------------------
    WvT = (Wv * g_kv[None, :]).T                             # [D_in, D_out]
    wvT_dev = np.ascontiguousarray(
        WvT.reshape(8, 128, D).transpose(1, 0, 2)).astype(bf16)   # [128, 8, 1024]
    woT_dev = np.ascontiguousarray(
        Wo.T.reshape(8, 128, D).transpose(1, 0, 2)).astype(bf16)  # [128, 8, 1024]
    WpT = (Wp * g_out[None, :]).T                            # [D, C]
    wpT_dev = np.ascontiguousarray(
        WpT.reshape(8, 128, C).transpose(1, 0, 2)).astype(bf16)   # [128, 8, 14]

    bo_comb = bo + Wo @ (Wv @ b_kv)                          # [D]
    bo_dev = np.ascontiguousarray(
        bo_comb.reshape(8, 128).T).astype(np.float32)        # [128, 8] = boT
    bp_comb = (bp + Wp @ b_out).reshape(C, 1).astype(np.float32)  # [14, 1]

    # --- token stream in both layouts ----------------------------------------
    # natural: order patches (0..N-1), cls, storage, pad
    tok = np.zeros((B, LTOT, D), dtype=bf16)
    tok[:, :N] = patches.astype(bf16)
    tok[:, N] = cls_tok.astype(bf16)
    tok[:, N + 1:N + 1 + S] = storage.astype(bf16)
    # natural layout stays token-major: the DMA rearrange makes 2 KB
    # per-partition descriptors, which interleave across partitions (16 KB
    # p-major bursts measurably stall engine SBUF reads)
    # transposed f8, p-major per-super contiguous:
    #   tokT_main[b, s, p, c, t] = tok[b, 1024 s + t, 128 c + p]
    tokf8 = tok[:, :N].astype(f8)                            # [B, N, D]
    tokT_main = np.ascontiguousarray(
        tokf8.reshape(B, NSUPER, SUPER, 8, 128).transpose(0, 1, 4, 3, 2))
    #   tokT_tail[b, p, c, t] = tok[b, N + t, 128 c + p]
    tokT_tail = np.ascontiguousarray(
        tok[:, N:].astype(f8).reshape(B, NTAIL, 8, 128).transpose(0, 3, 2, 1))

    weights = dict(wsc=wsc_dev, wvT=wvT_dev, woT=woT_dev, wpT=wpT_dev,
                   bo=bo_dev, bp=bp_comb)
    return tok, tokT_main, tokT_tail, weights


def _emit(tc, io):
    """Emit the Tile program for one core (BLOC batches)."""
    import concourse.bass as bass
    from concourse import mybir

    nc = tc.nc
    f32 = mybir.dt.float32
    bf16 = mybir.dt.bfloat16
    f8 = mybir.dt.float8e4
    AF = mybir.ActivationFunctionType
    OP = mybir.AluOpType

    toknat, tokTm, tokTt, wsc, wvT, woT, wpT, bo, bp, out = (
        io["toknat"], io["tokTm"], io["tokTt"], io["wsc"],
        io["wvT"], io["woT"], io["wpT"], io["bo"], io["bp"], io["out"])

    from contextlib import ExitStack
    ctx = ExitStack()
    with ctx:
        singles = ctx.enter_context(tc.tile_pool(name="singles", bufs=1))
        nat_pool = ctx.enter_context(tc.tile_pool(name="nat", bufs=5))
        tt_pool = ctx.enter_context(tc.tile_pool(name="tt", bufs=5))
        tl_pool = ctx.enter_context(tc.tile_pool(name="tl", bufs=2))
        small = ctx.enter_context(tc.tile_pool(name="small", bufs=3))
        ep_pool = ctx.enter_context(tc.tile_pool(name="ep", bufs=2))
        ps_small = ctx.enter_context(tc.tile_pool(name="ps_small", bufs=2, space="PSUM"))
        ps_mix = ctx.enter_context(tc.tile_pool(name="ps_mix", bufs=2, space="PSUM"))
        ps_den = ctx.enter_context(tc.tile_pool(name="ps_den", bufs=1, space="PSUM"))
        ps_tp = ctx.enter_context(tc.tile_pool(name="ps_tp", bufs=1, space="PSUM"))

        # ---- score weights (needed from the first super) ----------------
        wsc_sb = singles.tile([128, 8, 17], bf16)
        nc.sync.dma_start(wsc_sb[:], wsc[:])
        # tail weights are DMA'd once batch 0's token loads are queued
        wvT_sb = singles.tile([128, 8, D], bf16)
        woT_sb = singles.tile([128, 8, D], bf16)
        wpT_sb = singles.tile([128, 8, C], bf16)
        bo_sb = singles.tile([128, 8], f32)      # boT[p, k2] = bo_comb[128*k2+p]
        bp_sb = singles.tile([C, 1], f32)

        from concourse.masks import make_identity
        ident_b = singles.tile([128, 128], bf16)
        make_identity(nc, ident_b[:])
        onesf = singles.tile([128, 1], f32)
        nc.vector.memset(onesf[:], 1.0)
        ones_row = singles.tile([1, 128], f32)
        nc.vector.memset(ones_row[:], 1.0)

        # persistent per-core accumulators
        mixnT_all = singles.tile([128, 8, H, BLOC], bf16)    # [dp, c, h, b]

        x2junk = singles.tile([128, 1024], bf16)
        x2junk_a = singles.tile([128, 1024], bf16)

        def load_super(b, s):
            """Issue DMAs + x^2 + scores for (b, s); return state dict."""
            if s < NSUPER:
                natT = nat_pool.tile([128, NSUB, 1024], bf16, tag="nat")
                aux = nat_pool.tile([128, NSUB, 3], bf16, tag="aux")
                ttT = tt_pool.tile([128, 8, SUPER], f8, tag="tt")
                t0 = SUPER * s
                # split the nat load 6/2 across the HWDGE and SWDGE rings so
                # both carry ~1.5 MB/super (SDMA round-robins between rings)
                nc.sync.dma_start(
                    natT[:, 0:6, :],
                    toknat[b, t0:t0 + 768, :]
                    .rearrange("(j p) d -> p j d", p=128))
                nc.gpsimd.dma_start(ttT[:], tokTm[b, s])
                nc.gpsimd.dma_start(
                    natT[:, 6:8, :],
                    toknat[b, t0 + 768:t0 + 1024, :]
                    .rearrange("(j p) d -> p j d", p=128))
                nc.vector.memset(aux[:, :, 0:1], 1.0)
                nsub, npart = NSUB, 128
            else:
                natT = tl_pool.tile([NTAIL, 1, 1024], bf16, tag="natl")
                aux = tl_pool.tile([NTAIL, 1, 3], bf16, tag="auxl")
                ttT = tl_pool.tile([128, 8, NTAIL], f8, tag="ttl")
                nc.gpsimd.dma_start(ttT[:], tokTt[b])
                nc.sync.dma_start(
                    natT[:],
                    toknat[b, N:N + NTAIL, :].rearrange("(j p) d -> p j d", p=NTAIL))
                nc.vector.memset(aux[:, :, 0:1], 1.0)
                nsub, npart = 1, NTAIL

            # scores: s~[tok, h] (+ col16 = sum_d x) accumulated over 8 D-chunks
            scps = ps_small.tile([128, 8, 17], f32, tag="scps")
            ntok = 128 if s < NSUPER else NTAIL
            for j in range(nsub):
                for c in range(8):
                    nc.tensor.matmul(
                        scps[0:npart, j, :],
                        lhsT=ttT[:, c, 128 * j:128 * j + ntok],
                        rhs=wsc_sb[:, c, :],
                        start=(c == 0), stop=(c == 7))
            return dict(natT=natT, aux=aux, ttT=ttT, scps=scps,
                        nsub=nsub, npart=npart, s=s, b=b)

        def x2_super(st):
            """Per-token sum of squares, split DVE 3/8 + ACT 5/8.

            Emitted AFTER chain(prev) so that on DVE/ACT the previous super's
            chain never queues behind this super's (DMA-gated) squares."""
            nsub, npart, s, natT = st["nsub"], st["npart"], st["s"], st["natT"]
            x2acc = small.tile([128, 8], f32, tag="x2acc")
            for j in range(nsub):
                if s == NSUPER:
                    src = natT[:, j, 0:1024]     # [8, 1024]
                    nc.vector.scalar_tensor_tensor(
                        out=x2junk[0:npart, :], in0=src, scalar=1.0, in1=src,
                        op0=OP.mult, op1=OP.mult,
                        accum_out=x2acc[0:npart, j:j + 1])
                elif j in (0, 3, 6):
                    nc.vector.scalar_tensor_tensor(
                        out=x2junk[:], in0=natT[:, j, 0:1024], scalar=1.0,
                        in1=natT[:, j, 0:1024], op0=OP.mult, op1=OP.mult,
                        accum_out=x2acc[:, j:j + 1])
                else:
                    nc.scalar.activation(
                        x2junk_a[:], natT[:, j, 0:1024], AF.Square,
                        accum_out=x2acc[:, j:j + 1])
            st["x2acc"] = x2acc

        def chain_super(st):
            """LN stats -> r -> softmax weights for a loaded super (DVE/ACT)."""
            nsub, npart = st["nsub"], st["npart"]
            natT, aux, x2acc, scps = (st["natT"], st["aux"], st["x2acc"],
                                      st["scps"])
            sx = small.tile([128, 8], f32, tag="sx")
            nc.vector.tensor_copy(sx[0:npart, 0:nsub], scps[0:npart, 0:nsub, 16])
            # stash mu_t = sum(x)/1024 into aux column 1 (den matmul rhs)
            nc.vector.tensor_scalar_mul(aux[:, 0:nsub, 1:2],
                                        sx[0:npart, 0:nsub], 1.0 / 1024.0)
            v0 = small.tile([128, 8], f32, tag="v0")
            nc.vector.scalar_tensor_tensor(
                out=v0[0:npart, 0:nsub],
                in0=sx[0:npart, 0:nsub], scalar=-1.0 / (1024.0 * 1024.0),
                in1=sx[0:npart, 0:nsub], op0=OP.mult, op1=OP.mult)
            nc.vector.scalar_tensor_tensor(
                out=v0[0:npart, 0:nsub],
                in0=x2acc[0:npart, 0:nsub], scalar=1.0 / 1024.0,
                in1=v0[0:npart, 0:nsub], op0=OP.mult, op1=OP.add)
            nc.vector.tensor_scalar_add(v0[0:npart, 0:nsub],
                                        v0[0:npart, 0:nsub], EPS)
            # r = rsqrt(wn) via 2 Newton steps from seed 1.0 (wn ~= 1 +- 0.07)
            rr = small.tile([128, 8], f32, tag="rr")
            ra = small.tile([128, 8], f32, tag="ra")
            rc = small.tile([128, 8], f32, tag="rc")
            nc.vector.tensor_scalar(rr[0:npart, 0:nsub], v0[0:npart, 0:nsub],
                                    -0.5, 1.5, op0=OP.mult, op1=OP.add)
            nc.vector.scalar_tensor_tensor(
                out=ra[0:npart, 0:nsub], in0=rr[0:npart, 0:nsub], scalar=1.0,
                in1=rr[0:npart, 0:nsub], op0=OP.mult, op1=OP.mult)
            nc.vector.scalar_tensor_tensor(
                out=rc[0:npart, 0:nsub], in0=v0[0:npart, 0:nsub], scalar=-0.5,
                in1=ra[0:npart, 0:nsub], op0=OP.mult, op1=OP.mult)
            nc.vector.scalar_tensor_tensor(
                out=rr[0:npart, 0:nsub], in0=rc[0:npart, 0:nsub], scalar=1.5,
                in1=rr[0:npart, 0:nsub], op0=OP.add, op1=OP.mult)
            # stash 1/r = wn * r into aux column 2: the den matmul then
            # accumulates sum_t q/r = sum_t p from the q-columns alone
            nc.vector.scalar_tensor_tensor(
                out=aux[:, 0:nsub, 2:3], in0=v0[0:npart, 0:nsub],
                scalar=1.0, in1=rr[0:npart, 0:nsub], op0=OP.mult, op1=OP.mult)

            # s = s~ * r ; p = exp(s) ; q = p * r
            s_sb = small.tile([128, 8, H], f32, tag="s")
            nc.vector.scalar_tensor_tensor(
                out=s_sb[0:npart, 0:nsub, :],
                in0=scps[0:npart, 0:nsub, 0:16], scalar=1.0,
                in1=rr[0:npart, 0:nsub, None].broadcast_to([npart, nsub, H]),
                op0=OP.mult, op1=OP.mult)
            qp_sb = small.tile([128, 8, 2 * H], bf16, tag="qp")
            if st["s"] == NSUPER:
                # tail super: zero all q/p rows, then fill the real tokens
                np_ = NREAL - N                          # rows < np_ are real
                nc.vector.memset(qp_sb[0:npart, 0:nsub, :], 0.0)
                nc.scalar.activation(qp_sb[0:np_, 0:nsub, 16:32],
                                     s_sb[0:np_, 0:nsub, :], AF.Exp)
                nc.vector.scalar_tensor_tensor(
                    out=qp_sb[0:np_, 0:nsub, 0:16],
                    in0=qp_sb[0:np_, 0:nsub, 16:32], scalar=1.0,
                    in1=rr[0:np_, 0:nsub, None].broadcast_to([np_, nsub, H]),
                    op0=OP.mult, op1=OP.mult)
            else:
                nc.scalar.activation(qp_sb[:, 0:nsub, 16:32],
                                     s_sb[:, 0:nsub, :], AF.Exp)
                nc.vector.scalar_tensor_tensor(
                    out=qp_sb[:, 0:nsub, 0:16],
                    in0=qp_sb[:, 0:nsub, 16:32], scalar=1.0,
                    in1=rr[:, 0:nsub, None].broadcast_to([128, nsub, H]),
                    op0=OP.mult, op1=OP.mult)
            st["qp"] = qp_sb

        def mix_super(st, mixps, denps, first_mm, last_s):
            """Accumulate mix/den matmuls for a chained super.

            One 16-col stationary (the q columns) serves all three matmuls:
            denps cols are [sum q | sum q*mu | sum q/r] = [junk | c1 | den]."""
            nsub, npart = st["nsub"], st["npart"]
            natT, aux, qp_sb = st["natT"], st["aux"], st["qp"]
            for j in range(nsub):
                last = last_s and (j == nsub - 1)
                nc.tensor.matmul(
                    mixps[:, 0:512], lhsT=qp_sb[0:npart, j, 0:16],
                    rhs=natT[0:npart, j, 0:512], start=first_mm, stop=last)
                nc.tensor.matmul(
                    mixps[:, 512:1024], lhsT=qp_sb[0:npart, j, 0:16],
                    rhs=natT[0:npart, j, 512:1024], start=first_mm, stop=last)
                nc.tensor.matmul(
                    denps[:, 0:3], lhsT=qp_sb[0:npart, j, 0:16],
                    rhs=aux[0:npart, j, 0:3], start=first_mm, stop=last)
                first_mm = False

        def epilogue_batch(b, mixps, denps):
            """Per-batch: head mix -> mixnT_all[:, :, :, b]."""
            dinv = ep_pool.tile([H, 1], f32, tag="dinv")
            nc.vector.reciprocal(dinv[:], denps[:, 2:3])
            c1 = ep_pool.tile([H, 1], f32, tag="c1")
            nc.vector.tensor_copy(c1[:], denps[:, 1:2])
            mixn = ep_pool.tile([H, D], bf16, tag="mixn")
            nc.vector.scalar_tensor_tensor(
                out=mixn[:], in0=mixps[:], scalar=c1[:],
                in1=dinv[:, 0:1].broadcast_to([H, D]),
                op0=OP.subtract, op1=OP.mult)
            tp_all = ps_tp.tile([128, 8, H], bf16, tag="tp")
            for c in range(8):
                nc.tensor.transpose(tp_all[:, c, :], mixn[:, 128 * c:128 * c + 128],
                                    ident_b[0:H, 0:H])
            nc.vector.tensor_copy(mixnT_all[:, :, :, b], tp_all[:])

        def tail_half(b0, qpools):
            """Per-core tail for batches [b0, b0+4): Wv/Wo matmuls, out-LN, head.

            qpools: (pool, tag) per psum tile in allocation order
            [ctxps, poolps, sums, bcast, ops_] — each entry must be a slot
            that is provably free during this tail (see call sites)."""
            BL = 4
            qi = [0]

            def qtile(shape, dt_, nm):
                pool, tg = qpools[qi[0]]
                qi[0] += 1
                return pool.tile(shape, dt_, tag=tg, name=f"tl{b0}_{nm}")

            ctxT_sb = singles.tile([128, 8, BLOC], bf16, tag="ctxT")
            poolT_sb = singles.tile([128, 8, BL], f32, tag="poolT")
            sq_sb = singles.tile([128, 8, BL], f32, tag="sq")
            # ctxT[o, b] = sum_d Wv'[o, d] mixn[head(o), d]  (block-diag heads)
            ctxps = qtile([128, 8, BL], f32, "ctxps")
            for k in range(8):
                for half in range(2):
                    h = 2 * k + half
                    for c in range(8):
                        nc.tensor.matmul(
                            ctxps[64 * half:64 * half + 64, k, :],
                            lhsT=wvT_sb[:, c, 64 * h:64 * h + 64],
                            rhs=mixnT_all[:, c, h, b0:b0 + BL],
                            start=(c == 0), stop=(c == 7))
            nc.vector.tensor_copy(ctxT_sb[:, :, b0:b0 + BL], ctxps[:])
            # pooledT[o2, b] = sum_o Wo[o2, o] ctx[o, b] + boT
            poolps = qtile([128, 8, BL], f32, "poolps")
            for k2 in range(8):
                for k in range(8):
                    nc.tensor.matmul(
                        poolps[:, k2, :],
                        lhsT=woT_sb[:, k, 128 * k2:128 * k2 + 128],
                        rhs=ctxT_sb[:, k, b0:b0 + BL],
                        start=(k == 0), stop=(k == 7))
            nc.vector.scalar_tensor_tensor(
                out=poolT_sb[:], in0=poolps[:], scalar=1.0,
                in1=bo_sb[:, :, None].broadcast_to([128, 8, BL]),
                op0=OP.mult, op1=OP.add)
            nc.scalar.square(sq_sb[:], poolT_sb[:])
            # LN stats over the o2 (partition+chunk) axis via ones-matmuls
            sums = qtile([1, 2 * BL], f32, "sums")
            for k2 in range(8):
                nc.tensor.matmul(sums[0:1, 0:BL], lhsT=onesf[:, 0:1],
                                 rhs=poolT_sb[:, k2, :],
                                 start=(k2 == 0), stop=(k2 == 7))
            for k2 in range(8):
                nc.tensor.matmul(sums[0:1, BL:2 * BL], lhsT=onesf[:, 0:1],
                                 rhs=sq_sb[:, k2, :],
                                 start=(k2 == 0), stop=(k2 == 7))
            stats = singles.tile([1, 2 * BL], f32, tag="stats")
            nc.vector.tensor_copy(stats[:], sums[:])
            v8 = singles.tile([1, BL], f32, tag="v8")
            nc.vector.scalar_tensor_tensor(
                out=v8[:], in0=stats[0:1, 0:BL],
                scalar=-1.0 / (1024.0 * 1024.0),
                in1=stats[0:1, 0:BL], op0=OP.mult, op1=OP.mult)
            nc.vector.scalar_tensor_tensor(
                out=v8[:], in0=stats[0:1, BL:2 * BL], scalar=1.0 / 1024.0,
                in1=v8[:], op0=OP.mult, op1=OP.add)
            nc.vector.tensor_scalar_add(v8[:], v8[:], EPS)
            r8 = singles.tile([1, BL], f32, tag="r8")
            nc.vector.reciprocal(r8[:], v8[:])
            nc.scalar.sqrt(r8[:], r8[:])                     # rsqrt(var+eps)
            pair = singles.tile([1, 2 * BL], f32, tag="pair")  # [-mu*r | r]
            nc.vector.scalar_tensor_tensor(
                out=pair[0:1, 0:BL], in0=stats[0:1, 0:BL], scalar=-1.0 / 1024.0,
                in1=r8[:], op0=OP.mult, op1=OP.mult)
            nc.vector.tensor_copy(pair[0:1, BL:2 * BL], r8[:])
            bcast = qtile([128, 2 * BL], f32, "bcast")
            nc.tensor.matmul(bcast[:], lhsT=ones_row[0:1, :], rhs=pair[:],
                             start=True, stop=True)
            nr_bc = singles.tile([128, 1, 2 * BL], f32, tag="nrbc")
            nc.vector.tensor_copy(nr_bc[:, 0, :], bcast[:])

            # yhatT = (poolT - mu) * r in [o2, b] layout, then the head matmul
            yhatT = singles.tile([128, 8, BL], bf16, tag="yhat")
            tn = singles.tile([128, 8, BL], f32, tag="tn")
            nc.vector.scalar_tensor_tensor(
                out=tn[:], in0=poolT_sb[:], scalar=1.0,
                in1=nr_bc[:, 0:1, BL:2 * BL].broadcast_to([128, 8, BL]),
                op0=OP.mult, op1=OP.mult)
            nc.vector.scalar_tensor_tensor(
                out=yhatT[:], in0=tn[:], scalar=1.0,
                in1=nr_bc[:, 0:1, 0:BL].broadcast_to([128, 8, BL]),
                op0=OP.mult, op1=OP.add)
            ops_ = qtile([C, BL], f32, "ops")
            for c in range(8):
                nc.tensor.matmul(ops_[:], lhsT=wpT_sb[:, c, :],
                                 rhs=yhatT[:, c, :],
                                 start=(c == 0), stop=(c == 7))
            out_sb = singles.tile([C, BL], f32, tag="outsb")
            nc.vector.tensor_scalar(out_sb[:], ops_[:], bp_sb[:], None,
                                    op0=OP.add)
            nc.sync.dma_start(out[b0:b0 + BL, :].rearrange("b c -> c b"),
                              out_sb[:])

        # ================= main software-pipelined loop ===================
        # stream of (b, s) supers; PE does scores(i) then mix(i-1)
        stream = [(b, s) for b in range(BLOC) for s in range(NSUPER + 1)]
        SPB = NSUPER + 1                     # supers per batch
        prev = None                          # state of super i-1 (loaded)
        mixq = {}                            # batch -> [mixps, denps, first_mm]

        for idx, (b, s) in enumerate(stream):
            if s == 0:
                mixq[b] = [ps_mix.tile([H, D], f32, tag="mix", name=f"mixps{b}"),
                           ps_den.tile([H, 3], f32, tag="den", name=f"denps{b}"),
                           True]
            st = load_super(b, s)            # DMA + scores for (b, s)
            if prev is not None:
                chain_super(prev)            # DVE/ACT chain for i-1
                pb = prev["b"]
                m = mixq[pb]
                mix_super(prev, m[0], m[1], m[2], last_s=(prev["s"] == NSUPER))
                m[2] = False
            x2_super(st)                     # DVE/ACT squares for (b, s)
            if prev is not None:
                pb = prev["b"]
                if prev["s"] == NSUPER:      # batch pb fully accumulated
                    epilogue_batch(pb, m[0], m[1])
                    del mixq[pb]
                    if pb == 3:
                        # batch 3's mix slot was just released; batch 5's
                        # alloc comes later, so the tail may borrow it ONCE.
                        # Everything else funnels through the tp slot (the
                        # other mix slot and both scps slots are live, and
                        # borrowing them would deadlock DVE program order).
                        tail_half(0, [(ps_tp, "tp"), (ps_mix, "mix"),
                                      (ps_tp, "tp"), (ps_tp, "tp"),
                                      (ps_tp, "tp")])
            prev = st
            if idx == SPB + 2:
                # tail weights ride the (less loaded) SWDGE ring, late enough
                # not to starve the early nat loads
                nc.gpsimd.dma_start(wvT_sb[:], wvT[:])
                nc.gpsimd.dma_start(woT_sb[:], woT[:])
                nc.gpsimd.dma_start(wpT_sb[:], wpT[:])
                nc.gpsimd.dma_start(bo_sb[:], bo[:])
                nc.gpsimd.dma_start(bp_sb[:], bp[:])

        # drain the last super + last batch epilogue + second tail half
        chain_super(prev)
        m = mixq[prev["b"]]
        mix_super(prev, m[0], m[1], m[2], last_s=True)
        epilogue_batch(prev["b"], m[0], m[1])
        # the whole main loop is drained: every psum pool is free for the tail
        tail_half(4, [(ps_tp, "tp"), (ps_mix, "mix"), (ps_small, "scps"),
                      (ps_mix, "mix"), (ps_den, "den")])


def _build():
    import concourse.bass as bass
    import concourse.bacc as bacc
    import concourse.tile as tile
    from concourse import mybir

    f32 = mybir.dt.float32
    bf16 = mybir.dt.bfloat16

    nc = bacc.Bacc("TRN2", target_bir_lowering=False, debug=False,
                   num_devices=NCORES)
    io = {
        "toknat": nc.dram_tensor("toknat", [BLOC, LTOT, D], bf16,
                                 kind="ExternalInput").ap(),
        "tokTm": nc.dram_tensor("tokTm", [BLOC, NSUPER, 128, 8, SUPER],
                                mybir.dt.float8e4, kind="ExternalInput").ap(),
        "tokTt": nc.dram_tensor("tokTt", [BLOC, 128, 8, NTAIL],
                                mybir.dt.float8e4, kind="ExternalInput").ap(),
        "wsc": nc.dram_tensor("wsc", [128, 8, 17], bf16,
                              kind="ExternalInput").ap(),
        "wvT": nc.dram_tensor("wvT", [128, 8, D], bf16,
                              kind="ExternalInput").ap(),
        "woT": nc.dram_tensor("woT", [128, 8, D], bf16,
                              kind="ExternalInput").ap(),
        "wpT": nc.dram_tensor("wpT", [128, 8, C], bf16,
                              kind="ExternalInput").ap(),
        "bo": nc.dram_tensor("bo", [128, 8], f32, kind="ExternalInput").ap(),
        "bp": nc.dram_tensor("bp", [C, 1], f32, kind="ExternalInput").ap(),
        "out": nc.dram_tensor("out", [BLOC, C], f32,
                              kind="ExternalOutput").ap(),
    }
    with tile.TileContext(nc) as tc:
        _emit(tc, io)
    nc.compile()
    return nc


def _get_nc():
    if "nc" not in _cache:
        _cache["nc"] = _build()
    return _cache["nc"]


def run(inputs, trace=False, trace_kwargs=None):
    """Shard, run on 8 cores, gather.  Returns (out, BassKernelResults)."""
    from concourse.bass_utils import run_bass_kernel_spmd

    tok, tokTm, tokTt, weights = _host_prep(inputs)
    nc = _get_nc()
    in_maps = []
    for i in range(NCORES):
        m = dict(weights)
        m["toknat"] = np.ascontiguousarray(tok[i * BLOC:(i + 1) * BLOC])
        m["tokTm"] = np.ascontiguousarray(tokTm[i * BLOC:(i + 1) * BLOC])
        m["tokTt"] = np.ascontiguousarray(tokTt[i * BLOC:(i + 1) * BLOC])
        in_maps.append(m)
    res = run_bass_kernel_spmd(nc, in_maps, core_ids=list(range(NCORES)),
                               trace=trace, **(trace_kwargs or {}))
    out = np.concatenate([np.asarray(res.results[i]["out"], dtype=np.float32)
                          for i in range(NCORES)], axis=0)
    return out, res


def kernel(**inputs):
    out, _ = run(inputs)
    return out


# revision 46
# speedup vs baseline: 1.1680x; 1.0499x over previous
"""AttentionPoolHead Trainium2 kernel (8 NeuronCores, batch-data-parallel).

Takes FULL inputs (as produced by setup_inputs), returns FULL (B, C) output.

Math (exact rewrite of the reference):
  tokens = [patches | cls | storage | zero-pad]            (order irrelevant: softmax-pool
                                                            is permutation invariant)
  kv     = LN(tokens) ; q fixed vector  =>  per-token score collapses to
      s[t,h] = r_t * (x_t . w''[:,h])
  with w'' = (Wk_head.T @ qp) * g / sqrt(HD), mean-centered over D (the -mu term of LN
  and all per-head constants vanish under softmax).  With q_t = p_t * r_t,
  p_t = exp(s_t):
      head_mix_h = ((sum_t q_t x_t) - rowmean correction) / (sum_t p_t)
      ctx = Wv' @ head_mix (+ folded biases), pooled = Wo @ ctx + bo', LN, Wp'' head.

Pipeline notes (v2):
  - token stream is read twice: bf16 natural layout (mix matmul rhs) and f8
    transposed per-super-contiguous layout (score matmul stationary).
  - per-token sum(x^2) is split across ACT (4/8), DVE (2/8), GPSIMD (2/8).
  - PE emission is software-pipelined one super: scores(s) ; mix(s-1), so the
    DVE score->qp chain of super s-1 overlaps the score matmuls of super s.
  - the per-core tail runs twice on batch halves (0-3 after batch 3, 4-7 at
    the end) so most of it hides under the second half's token streaming.
"""

import numpy as np

B, S, N, D, H, C = 64, 4, 4096, 1024, 16, 14
HD = D // H
EPS = 1e-5
NCORES = 8
BLOC = B // NCORES          # batches per core
NREAL = 1 + S + N           # 4101 real tokens
NTAIL = 8                   # tail mini-super: 5 real + 3 zero-pad tokens
LTOT = N + NTAIL            # 4104 tokens kept per batch
NSUPER = 4                  # full supers of 1024 tokens (the patches region)
SUPER = 1024
NSUB = 8                    # 128-token subtiles per full super

_cache = {}


def _f32(x):
    return np.ascontiguousarray(np.asarray(x, dtype=np.float32))


def _host_prep(inputs):
    """All weight folding + token layout prep on the host (numpy)."""
    import ml_dtypes

    bf16 = ml_dtypes.bfloat16
    f8 = ml_dtypes.float8_e4m3

    cls_tok = _f32(inputs["cls_tok"])        # [B, D]
    storage = _f32(inputs["storage"])        # [B, S, D]
    patches = _f32(inputs["patches"])        # [B, N, D]
    query = _f32(inputs["query"]).reshape(D)
    g_kv = _f32(inputs["ln_kv_g"])
    b_kv = _f32(inputs["ln_kv_b"])
    Wq = _f32(inputs["Wq"]); Wk = _f32(inputs["Wk"]); Wv = _f32(inputs["Wv"])
    bq = _f32(inputs["bq"])
    Wo = _f32(inputs["Wo"]); bo = _f32(inputs["bo"])
    g_out = _f32(inputs["ln_out_g"]); b_out = _f32(inputs["ln_out_b"])
    Wp = _f32(inputs["Wp"]); bp = _f32(inputs["bp"])

    # --- score weights: s[t,h] = r_t * (x_t . w''[:, h]) ----------------------
    qp = query @ Wq.T + bq                                   # [D]
    w_raw = np.einsum("hid,hi->dh", Wk.reshape(H, HD, D), qp.reshape(H, HD))
    w_raw /= np.sqrt(HD).astype(np.float32)
    wpr = w_raw * g_kv[:, None]                              # fold LN gain
    wpp = wpr - wpr.mean(0, keepdims=True)                   # fold LN mean-centering
    # device layout [128, 8, 17]: [:, c, 0:16] = wpp[128c+p, :], col 16 = ones
    wsc_dev = np.zeros((128, 8, 17), dtype=np.float32)
    wsc_dev[:, :, :16] = wpp.reshape(8, 128, 16).transpose(1, 0, 2)
    wsc_dev[:, :, 16] = 1.0
    wsc_dev = wsc_dev.astype(bf16)

    # --- Wv / Wo / Wp folds ---------------------------------------------------
    WvT = (Wv * g_kv[None, :]).T                             # [D_in, D_out]
    wvT_dev = np.ascontiguousarray(
        WvT.reshape(8, 128, D).transpose(1, 0, 2)).astype(bf16)   # [128, 8, 1024]
    woT_dev = np.ascontiguousarray(
        Wo.T.reshape(8, 128, D).transpose(1, 0, 2)).astype(bf16)  # [128, 8, 1024]
    WpT = (Wp * g_out[None, :]).T                            # [D, C]
    wpT_dev = np.ascontiguousarray(
        WpT.reshape(8, 128, C).transpose(1, 0, 2)).astype(bf16)   # [128, 8, 14]

    bo_comb = bo + Wo @ (Wv @ b_kv)                          # [D]
    bo_dev = np.ascontiguousarray(
        bo_comb.reshape(8, 128).T).astype(np.float32)        # [128, 8] = boT
    bp_comb = (bp + Wp @ b_out).reshape(C, 1).astype(np.float32)  # [14, 1]

    # --- token stream in both layouts ----------------------------------------
    # natural: order patches (0..N-1), cls, storage, pad
    tok = np.zeros((B, LTOT, D), dtype=bf16)
    tok[:, :N] = patches.astype(bf16)
    tok[:, N] = cls_tok.astype(bf16)
    tok[:, N + 1:N + 1 + S] = storage.astype(bf16)
    # natural layout stays token-major: the DMA rearrange makes 2 KB
    # per-partition descriptors, which interleave across partitions (16 KB
    # p-major bursts measurably stall engine SBUF reads)
    # transposed f8, p-major per-super contiguous:
    #   tokT_main[b, s, p, c, t] = tok[b, 1024 s + t, 128 c + p]
    tokf8 = tok[:, :N].astype(f8)                            # [B, N, D]
    tokT_main = np.ascontiguousarray(
        tokf8.reshape(B, NSUPER, SUPER, 8, 128).transpose(0, 1, 4, 3, 2))
    #   tokT_tail[b, p, c, t] = tok[b, N + t, 128 c + p]
    tokT_tail = np.ascontiguousarray(
        tok[:, N:].astype(f8).reshape(B, NTAIL, 8, 128).transpose(0, 3, 2, 1))

    weights = dict(wsc=wsc_dev, wvT=wvT_dev, woT=woT_dev, wpT=wpT_dev,
                   bo=bo_dev, bp=bp_comb)
    return tok, tokT_main, tokT_tail, weights


def _emit(tc, io):
    """Emit the Tile program for one core (BLOC batches)."""
    import concourse.bass as bass
    from concourse import mybir

    nc = tc.nc
    f32 = mybir.dt.float32
    bf16 = mybir.dt.bfloat16
    f8 = mybir.dt.float8e4
    AF = mybir.ActivationFunctionType
    OP = mybir.AluOpType

    toknat, tokTm, tokTt, wsc, wvT, woT, wpT, bo, bp, out = (
        io["toknat"], io["tokTm"], io["tokTt"], io["wsc"],
        io["wvT"], io["woT"], io["wpT"], io["bo"], io["bp"], io["out"])

    from contextlib import ExitStack
    ctx = ExitStack()
    with ctx:
        singles = ctx.enter_context(tc.tile_pool(name="singles", bufs=1))
        nat_pool = ctx.enter_context(tc.tile_pool(name="nat", bufs=5))
        tt_pool = ctx.enter_context(tc.tile_pool(name="tt", bufs=5))
        tl_pool = ctx.enter_context(tc.tile_pool(name="tl", bufs=2))
        small = ctx.enter_context(tc.tile_pool(name="small", bufs=3))
        ep_pool = ctx.enter_context(tc.tile_pool(name="ep", bufs=2))
        ps_small = ctx.enter_context(tc.tile_pool(name="ps_small", bufs=2, space="PSUM"))
        ps_mix = ctx.enter_context(tc.tile_pool(name="ps_mix", bufs=2, space="PSUM"))
        ps_den = ctx.enter_context(tc.tile_pool(name="ps_den", bufs=1, space="PSUM"))
        ps_tp = ctx.enter_context(tc.tile_pool(name="ps_tp", bufs=1, space="PSUM"))

        # ---- score weights (needed from the first super) ----------------
        wsc_sb = singles.tile([128, 8, 17], bf16)
        nc.sync.dma_start(wsc_sb[:], wsc[:])
        # tail weights are DMA'd once batch 0's token loads are queued
        wvT_sb = singles.tile([128, 8, D], bf16)
        woT_sb = singles.tile([128, 8, D], bf16)
        wpT_sb = singles.tile([128, 8, C], bf16)
        bo_sb = singles.tile([128, 8], f32)      # boT[p, k2] = bo_comb[128*k2+p]
        bp_sb = singles.tile([C, 1], f32)

        from concourse.masks import make_identity
        ident_b = singles.tile([128, 128], bf16)
        make_identity(nc, ident_b[:])
        onesf = singles.tile([128, 1], f32)
        nc.vector.memset(onesf[:], 1.0)
        ones_row = singles.tile([1, 128], f32)
        nc.vector.memset(ones_row[:], 1.0)

        # persistent per-core accumulators
        mixnT_all = singles.tile([128, 8, H, BLOC], bf16)    # [dp, c, h, b]

        x2junk = singles.tile([128, 1024], bf16)
        x2junk_a = singles.tile([128, 1024], bf16)

        def load_super(b, s):
            """Issue DMAs + x^2 + scores for (b, s); return state dict."""
            if s < NSUPER:
                natT = nat_pool.tile([128, NSUB, 1024], bf16, tag="nat")
                aux = nat_pool.tile([128, NSUB, 3], bf16, tag="aux")
                ttT = tt_pool.tile([128, 8, SUPER], f8, tag="tt")
                t0 = SUPER * s
                # single HWDGE ring: FIFO order auto-balances the two streams
                # (SWDGE descriptor-ring fetches contend with data on the SBUF
                # AXI ports and cost ~10% aggregate DMA bandwidth)
                nc.sync.dma_start(ttT[:], tokTm[b, s])
                nc.sync.dma_start(
                    natT[:],
                    toknat[b, t0:t0 + SUPER, :]
                    .rearrange("(j p) d -> p j d", p=128))
                nc.vector.memset(aux[:, :, 0:1], 1.0)
                nsub, npart = NSUB, 128
            else:
                natT = tl_pool.tile([NTAIL, 1, 1024], bf16, tag="natl")
                aux = tl_pool.tile([NTAIL, 1, 3], bf16, tag="auxl")
                ttT = tl_pool.tile([128, 8, NTAIL], f8, tag="ttl")
                nc.sync.dma_start(ttT[:], tokTt[b])
                nc.sync.dma_start(
                    natT[:],
                    toknat[b, N:N + NTAIL, :].rearrange("(j p) d -> p j d", p=NTAIL))
                nc.vector.memset(aux[:, :, 0:1], 1.0)
                nsub, npart = 1, NTAIL

            # scores: s~[tok, h] (+ col16 = sum_d x) accumulated over 8 D-chunks
            scps = ps_small.tile([128, 8, 17], f32, tag="scps")
            ntok = 128 if s < NSUPER else NTAIL
            for j in range(nsub):
                for c in range(8):
                    nc.tensor.matmul(
                        scps[0:npart, j, :],
                        lhsT=ttT[:, c, 128 * j:128 * j + ntok],
                        rhs=wsc_sb[:, c, :],
                        start=(c == 0), stop=(c == 7))
            return dict(natT=natT, aux=aux, ttT=ttT, scps=scps,
                        nsub=nsub, npart=npart, s=s, b=b)

        def x2_super(st):
            """Per-token sum of squares, split DVE 3/8 + ACT 5/8.

            Emitted AFTER chain(prev) so that on DVE/ACT the previous super's
            chain never queues behind this super's (DMA-gated) squares."""
            nsub, npart, s, natT = st["nsub"], st["npart"], st["s"], st["natT"]
            x2acc = small.tile([128, 8], f32, tag="x2acc")
            for j in range(nsub):
                if s == NSUPER:
                    src = natT[:, j, 0:1024]     # [8, 1024]
                    nc.vector.scalar_tensor_tensor(
                        out=x2junk[0:npart, :], in0=src, scalar=1.0, in1=src,
                        op0=OP.mult, op1=OP.mult,
                        accum_out=x2acc[0:npart, j:j + 1])
                elif j in (0, 3, 6):
                    nc.vector.scalar_tensor_tensor(
                        out=x2junk[:], in0=natT[:, j, 0:1024], scalar=1.0,
                        in1=natT[:, j, 0:1024], op0=OP.mult, op1=OP.mult,
                        accum_out=x2acc[:, j:j + 1])
                else:
                    nc.scalar.activation(
                        x2junk_a[:], natT[:, j, 0:1024], AF.Square,
                        accum_out=x2acc[:, j:j + 1])
            st["x2acc"] = x2acc

        def chain_super(st):
            """LN stats -> r -> softmax weights for a loaded super (DVE/ACT)."""
            nsub, npart = st["nsub"], st["npart"]
            natT, aux, x2acc, scps = (st["natT"], st["aux"], st["x2acc"],
                                      st["scps"])
            sx = small.tile([128, 8], f32, tag="sx")
            nc.vector.tensor_copy(sx[0:npart, 0:nsub], scps[0:npart, 0:nsub, 16])
            # stash mu_t = sum(x)/1024 into aux column 1 (den matmul rhs)
            nc.vector.tensor_scalar_mul(aux[:, 0:nsub, 1:2],
                                        sx[0:npart, 0:nsub], 1.0 / 1024.0)
            v0 = small.tile([128, 8], f32, tag="v0")
            nc.vector.scalar_tensor_tensor(
                out=v0[0:npart, 0:nsub],
                in0=sx[0:npart, 0:nsub], scalar=-1.0 / (1024.0 * 1024.0),
                in1=sx[0:npart, 0:nsub], op0=OP.mult, op1=OP.mult)
            nc.vector.scalar_tensor_tensor(
                out=v0[0:npart, 0:nsub],
                in0=x2acc[0:npart, 0:nsub], scalar=1.0 / 1024.0,
                in1=v0[0:npart, 0:nsub], op0=OP.mult, op1=OP.add)
            nc.vector.tensor_scalar_add(v0[0:npart, 0:nsub],
                                        v0[0:npart, 0:nsub], EPS)
            # r = rsqrt(wn) via 2 Newton steps from seed 1.0 (wn ~= 1 +- 0.07)
            rr = small.tile([128, 8], f32, tag="rr")
            ra = small.tile([128, 8], f32, tag="ra")
            rc = small.tile([128, 8], f32, tag="rc")
            nc.vector.tensor_scalar(rr[0:npart, 0:nsub], v0[0:npart, 0:nsub],
                                    -0.5, 1.5, op0=OP.mult, op1=OP.add)
            nc.vector.scalar_tensor_tensor(
                out=ra[0:npart, 0:nsub], in0=rr[0:npart, 0:nsub], scalar=1.0,
                in1=rr[0:npart, 0:nsub], op0=OP.mult, op1=OP.mult)
            nc.vector.scalar_tensor_tensor(
                out=rc[0:npart, 0:nsub], in0=v0[0:npart, 0:nsub], scalar=-0.5,
                in1=ra[0:npart, 0:nsub], op0=OP.mult, op1=OP.mult)
            nc.vector.scalar_tensor_tensor(
                out=rr[0:npart, 0:nsub], in0=rc[0:npart, 0:nsub], scalar=1.5,
                in1=rr[0:npart, 0:nsub], op0=OP.add, op1=OP.mult)
            # stash 1/r = wn * r into aux column 2: the den matmul then
            # accumulates sum_t q/r = sum_t p from the q-columns alone
            nc.vector.scalar_tensor_tensor(
                out=aux[:, 0:nsub, 2:3], in0=v0[0:npart, 0:nsub],
                scalar=1.0, in1=rr[0:npart, 0:nsub], op0=OP.mult, op1=OP.mult)

            # s = s~ * r ; p = exp(s) ; q = p * r
            s_sb = small.tile([128, 8, H], f32, tag="s")
            nc.vector.scalar_tensor_tensor(
                out=s_sb[0:npart, 0:nsub, :],
                in0=scps[0:npart, 0:nsub, 0:16], scalar=1.0,
                in1=rr[0:npart, 0:nsub, None].broadcast_to([npart, nsub, H]),
                op0=OP.mult, op1=OP.mult)
            qp_sb = small.tile([128, 8, 2 * H], bf16, tag="qp")
            if st["s"] == NSUPER:
                # tail super: zero all q/p rows, then fill the real tokens
                np_ = NREAL - N                          # rows < np_ are real
                nc.vector.memset(qp_sb[0:npart, 0:nsub, :], 0.0)
                nc.scalar.activation(qp_sb[0:np_, 0:nsub, 16:32],
                                     s_sb[0:np_, 0:nsub, :], AF.Exp)
                nc.vector.scalar_tensor_tensor(
                    out=qp_sb[0:np_, 0:nsub, 0:16],
                    in0=qp_sb[0:np_, 0:nsub, 16:32], scalar=1.0,
                    in1=rr[0:np_, 0:nsub, None].broadcast_to([np_, nsub, H]),
                    op0=OP.mult, op1=OP.mult)
            else:
                nc.scalar.activation(qp_sb[:, 0:nsub, 16:32],
                                     s_sb[:, 0:nsub, :], AF.Exp)
                nc.vector.scalar_tensor_tensor(
                    out=qp_sb[:, 0:nsub, 0:16],
                    in0=qp_sb[:, 0:nsub, 16:32], scalar=1.0,
                    in1=rr[:, 0:nsub, None].broadcast_to([128, nsub, H]),
                    op0=OP.mult, op1=OP.mult)
            st["qp"] = qp_sb

        def mix_super(st, mixps, denps, first_mm, last_s):
            """Accumulate mix/den matmuls for a chained super.

            One 16-col stationary (the q columns) serves all three matmuls:
            denps cols are [sum q | sum q*mu | sum q/r] = [junk | c1 | den]."""
            nsub, npart = st["nsub"], st["npart"]
            natT, aux, qp_sb = st["natT"], st["aux"], st["qp"]
            for j in range(nsub):
                last = last_s and (j == nsub - 1)
                nc.tensor.matmul(
                    mixps[:, 0:512], lhsT=qp_sb[0:npart, j, 0:16],
                    rhs=natT[0:npart, j, 0:512], start=first_mm, stop=last)
                nc.tensor.matmul(
                    mixps[:, 512:1024], lhsT=qp_sb[0:npart, j, 0:16],
                    rhs=natT[0:npart, j, 512:1024], start=first_mm, stop=last)
                nc.tensor.matmul(
                    denps[:, 0:3], lhsT=qp_sb[0:npart, j, 0:16],
                    rhs=aux[0:npart, j, 0:3], start=first_mm, stop=last)
                first_mm = False

        def epilogue_batch(b, mixps, denps):
            """Per-batch: head mix -> mixnT_all[:, :, :, b]."""
            dinv = ep_pool.tile([H, 1], f32, tag="dinv")
            nc.vector.reciprocal(dinv[:], denps[:, 2:3])
            c1 = ep_pool.tile([H, 1], f32, tag="c1")
            nc.vector.tensor_copy(c1[:], denps[:, 1:2])
            mixn = ep_pool.tile([H, D], bf16, tag="mixn")
            nc.vector.scalar_tensor_tensor(
                out=mixn[:], in0=mixps[:], scalar=c1[:],
                in1=dinv[:, 0:1].broadcast_to([H, D]),
                op0=OP.subtract, op1=OP.mult)
            tp_all = ps_tp.tile([128, 8, H], bf16, tag="tp")
            for c in range(8):
                nc.tensor.transpose(tp_all[:, c, :], mixn[:, 128 * c:128 * c + 128],
                                    ident_b[0:H, 0:H])
            nc.vector.tensor_copy(mixnT_all[:, :, :, b], tp_all[:])

        def tail_half(b0, qpools):
            """Per-core tail for batches [b0, b0+4): Wv/Wo matmuls, out-LN, head.

            qpools: (pool, tag) per psum tile in allocation order
            [ctxps, poolps, sums, bcast, ops_] — each entry must be a slot
            that is provably free during this tail (see call sites)."""
            BL = 4
            qi = [0]

            def qtile(shape, dt_, nm):
                pool, tg = qpools[qi[0]]
                qi[0] += 1
                return pool.tile(shape, dt_, tag=tg, name=f"tl{b0}_{nm}")

            ctxT_sb = singles.tile([128, 8, BLOC], bf16, tag="ctxT")
            poolT_sb = singles.tile([128, 8, BL], f32, tag="poolT")
            sq_sb = singles.tile([128, 8, BL], f32, tag="sq")
            # ctxT[o, b] = sum_d Wv'[o, d] mixn[head(o), d]  (block-diag heads)
            ctxps = qtile([128, 8, BL], f32, "ctxps")
            for k in range(8):
                for half in range(2):
                    h = 2 * k + half
                    for c in range(8):
                        nc.tensor.matmul(
                            ctxps[64 * half:64 * half + 64, k, :],
                            lhsT=wvT_sb[:, c, 64 * h:64 * h + 64],
                            rhs=mixnT_all[:, c, h, b0:b0 + BL],
                            start=(c == 0), stop=(c == 7))
            nc.vector.tensor_copy(ctxT_sb[:, :, b0:b0 + BL], ctxps[:])
            # pooledT[o2, b] = sum_o Wo[o2, o] ctx[o, b] + boT
            poolps = qtile([128, 8, BL], f32, "poolps")
            for k2 in range(8):
                for k in range(8):
                    nc.tensor.matmul(
                        poolps[:, k2, :],
                        lhsT=woT_sb[:, k, 128 * k2:128 * k2 + 128],
                        rhs=ctxT_sb[:, k, b0:b0 + BL],
                        start=(k == 0), stop=(k == 7))
            nc.vector.scalar_tensor_tensor(
                out=poolT_sb[:], in0=poolps[:], scalar=1.0,
                in1=bo_sb[:, :, None].broadcast_to([128, 8, BL]),
                op0=OP.mult, op1=OP.add)
            nc.scalar.square(sq_sb[:], poolT_sb[:])
            # LN stats over the o2 (partition+chunk) axis via ones-matmuls
            sums = qtile([1, 2 * BL], f32, "sums")
            for k2 in range(8):
                nc.tensor.matmul(sums[0:1, 0:BL], lhsT=onesf[:, 0:1],
                                 rhs=poolT_sb[:, k2, :],
                                 start=(k2 == 0), stop=(k2 == 7))
            for k2 in range(8):
                nc.tensor.matmul(sums[0:1, BL:2 * BL], lhsT=onesf[:, 0:1],
                                 rhs=sq_sb[:, k2, :],
                                 start=(k2 == 0), stop=(k2 == 7))
            stats = singles.tile([1, 2 * BL], f32, tag="stats")
            nc.vector.tensor_copy(stats[:], sums[:])
            v8 = singles.tile([1, BL], f32, tag="v8")
            nc.vector.scalar_tensor_tensor(
                out=v8[:], in0=stats[0:1, 0:BL],
                scalar=-1.0 / (1024.0 * 1024.0),
                in1=stats[0:1, 0:BL], op0=OP.mult, op1=OP.mult)
            nc.vector.scalar_tensor_tensor(
                out=v8[:], in0=stats[0:1, BL:2 * BL], scalar=1.0 / 1024.0,
                in1=v8[:], op0=OP.mult, op1=OP.add)
            nc.vector.tensor_scalar_add(v8[:], v8[:], EPS)
            r8 = singles.tile([1, BL], f32, tag="r8")
            nc.vector.reciprocal(r8[:], v8[:])
            nc.scalar.sqrt(r8[:], r8[:])                     # rsqrt(var+eps)
            pair = singles.tile([1, 2 * BL], f32, tag="pair")  # [-mu*r | r]
            nc.vector.scalar_tensor_tensor(
                out=pair[0:1, 0:BL], in0=stats[0:1, 0:BL], scalar=-1.0 / 1024.0,
                in1=r8[:], op0=OP.mult, op1=OP.mult)
            nc.vector.tensor_copy(pair[0:1, BL:2 * BL], r8[:])
            bcast = qtile([128, 2 * BL], f32, "bcast")
            nc.tensor.matmul(bcast[:], lhsT=ones_row[0:1, :], rhs=pair[:],
                             start=True, stop=True)
            nr_bc = singles.tile([128, 1, 2 * BL], f32, tag="nrbc")
            nc.vector.tensor_copy(nr_bc[:, 0, :], bcast[:])

            # yhatT = (poolT - mu) * r in [o2, b] layout, then the head matmul
            yhatT = singles.tile([128, 8, BL], bf16, tag="yhat")
            tn = singles.tile([128, 8, BL], f32, tag="tn")
            nc.vector.scalar_tensor_tensor(
                out=tn[:], in0=poolT_sb[:], scalar=1.0,
                in1=nr_bc[:, 0:1, BL:2 * BL].broadcast_to([128, 8, BL]),
                op0=OP.mult, op1=OP.mult)
            nc.vector.scalar_tensor_tensor(
                out=yhatT[:], in0=tn[:], scalar=1.0,
                in1=nr_bc[:, 0:1, 0:BL].broadcast_to([128, 8, BL]),
                op0=OP.mult, op1=OP.add)
            ops_ = qtile([C, BL], f32, "ops")
            for c in range(8):
                nc.tensor.matmul(ops_[:], lhsT=wpT_sb[:, c, :],
                                 rhs=yhatT[:, c, :],
                                 start=(c == 0), stop=(c == 7))
            out_sb = singles.tile([C, BL], f32, tag="outsb")
            nc.vector.tensor_scalar(out_sb[:], ops_[:], bp_sb[:], None,
                                    op0=OP.add)
            nc.sync.dma_start(out[b0:b0 + BL, :].rearrange("b c -> c b"),
                              out_sb[:])

        # ================= main software-pipelined loop ===================
        # stream of (b, s) supers; PE does scores(i) then mix(i-1)
        stream = [(b, s) for b in range(BLOC) for s in range(NSUPER + 1)]
        SPB = NSUPER + 1                     # supers per batch
        prev = None                          # state of super i-1 (loaded)
        mixq = {}                            # batch -> [mixps, denps, first_mm]

        for idx, (b, s) in enumerate(stream):
            if s == 0:
                mixq[b] = [ps_mix.tile([H, D], f32, tag="mix", name=f"mixps{b}"),
                           ps_den.tile([H, 3], f32, tag="den", name=f"denps{b}"),
                           True]
            st = load_super(b, s)            # DMA + scores for (b, s)
            if prev is not None:
                chain_super(prev)            # DVE/ACT chain for i-1
                pb = prev["b"]
                m = mixq[pb]
                mix_super(prev, m[0], m[1], m[2], last_s=(prev["s"] == NSUPER))
                m[2] = False
            x2_super(st)                     # DVE/ACT squares for (b, s)
            if prev is not None:
                pb = prev["b"]
                if prev["s"] == NSUPER:      # batch pb fully accumulated
                    epilogue_batch(pb, m[0], m[1])
                    del mixq[pb]
                    if pb == 3:
                        # batch 3's mix slot was just released; batch 5's
                        # alloc comes later, so the tail may borrow it ONCE.
                        # Everything else funnels through the tp slot (the
                        # other mix slot and both scps slots are live, and
                        # borrowing them would deadlock DVE program order).
                        tail_half(0, [(ps_tp, "tp"), (ps_mix, "mix"),
                                      (ps_tp, "tp"), (ps_tp, "tp"),
                                      (ps_tp, "tp")])
            prev = st
            if idx == SPB + 2:
                # tail weights queued mid-batch-1: the prefetch window has
                # enough slack there to absorb their ~12us of ring time
                nc.sync.dma_start(wvT_sb[:], wvT[:])
                nc.sync.dma_start(woT_sb[:], woT[:])
                nc.sync.dma_start(wpT_sb[:], wpT[:])
                nc.sync.dma_start(bo_sb[:], bo[:])
                nc.sync.dma_start(bp_sb[:], bp[:])

        # drain the last super + last batch epilogue + second tail half
        chain_super(prev)
        m = mixq[prev["b"]]
        mix_super(prev, m[0], m[1], m[2], last_s=True)
        epilogue_batch(prev["b"], m[0], m[1])
        # the whole main loop is drained: every psum pool is free for the tail
        tail_half(4, [(ps_tp, "tp"), (ps_mix, "mix"), (ps_small, "scps"),
                      (ps_mix, "mix"), (ps_den, "den")])


def _build():
    import concourse.bass as bass
    import concourse.bacc as bacc
    import concourse.tile as tile
    from concourse import mybir

    f32 = mybir.dt.float32
    bf16 = mybir.dt.bfloat16

    nc = bacc.Bacc("TRN2", target_bir_lowering=False, debug=False,
                   num_devices=NCORES)
    io = {
        "toknat": nc.dram_tensor("toknat", [BLOC, LTOT, D], bf16,
                                 kind="ExternalInput").ap(),
        "tokTm": nc.dram_tensor("tokTm", [BLOC, NSUPER, 128, 8, SUPER],
                                mybir.dt.float8e4, kind="ExternalInput").ap(),
        "tokTt": nc.dram_tensor("tokTt", [BLOC, 128, 8, NTAIL],
                                mybir.dt.float8e4, kind="ExternalInput").ap(),
        "wsc": nc.dram_tensor("wsc", [128, 8, 17], bf16,
                              kind="ExternalInput").ap(),
        "wvT": nc.dram_tensor("wvT", [128, 8, D], bf16,
                              kind="ExternalInput").ap(),
        "woT": nc.dram_tensor("woT", [128, 8, D], bf16,
                              kind="ExternalInput").ap(),
        "wpT": nc.dram_tensor("wpT", [128, 8, C], bf16,
                              kind="ExternalInput").ap(),
        "bo": nc.dram_tensor("bo", [128, 8], f32, kind="ExternalInput").ap(),
        "bp": nc.dram_tensor("bp", [C, 1], f32, kind="ExternalInput").ap(),
        "out": nc.dram_tensor("out", [BLOC, C], f32,
                              kind="ExternalOutput").ap(),
    }
    with tile.TileContext(nc) as tc:
        _emit(tc, io)
    nc.compile()
    return nc


def _get_nc():
    if "nc" not in _cache:
        _cache["nc"] = _build()
    return _cache["nc"]


def run(inputs, trace=False, trace_kwargs=None):
    """Shard, run on 8 cores, gather.  Returns (out, BassKernelResults)."""
    from concourse.bass_utils import run_bass_kernel_spmd

    tok, tokTm, tokTt, weights = _host_prep(inputs)
    nc = _get_nc()
    in_maps = []
    for i in range(NCORES):
        m = dict(weights)
        m["toknat"] = np.ascontiguousarray(tok[i * BLOC:(i + 1) * BLOC])
        m["tokTm"] = np.ascontiguousarray(tokTm[i * BLOC:(i + 1) * BLOC])
        m["tokTt"] = np.ascontiguousarray(tokTt[i * BLOC:(i + 1) * BLOC])
        in_maps.append(m)
    res = run_bass_kernel_spmd(nc, in_maps, core_ids=list(range(NCORES)),
                               trace=trace, **(trace_kwargs or {}))
    out = np.concatenate([np.asarray(res.results[i]["out"], dtype=np.float32)
                          for i in range(NCORES)], axis=0)
    return out, res


def kernel(**inputs):
    out, _ = run(inputs)
    return out


# revision 57
# speedup vs baseline: 1.1926x; 1.0211x over previous
"""AttentionPoolHead Trainium2 kernel (8 NeuronCores, batch-data-parallel).

Takes FULL inputs (as produced by setup_inputs), returns FULL (B, C) output.

Math (exact rewrite of the reference):
  tokens = [patches | cls | storage | zero-pad]            (order irrelevant: softmax-pool
                                                            is permutation invariant)
  kv     = LN(tokens) ; q fixed vector  =>  per-token score collapses to
      s[t,h] = r_t * (x_t . w''[:,h])
  with w'' = (Wk_head.T @ qp) * g / sqrt(HD), mean-centered over D (the -mu term of LN
  and all per-head constants vanish under softmax).  With q_t = p_t * r_t,
  p_t = exp(s_t):
      head_mix_h = ((sum_t q_t x_t) - rowmean correction) / (sum_t p_t)
      ctx = Wv' @ head_mix (+ folded biases), pooled = Wo @ ctx + bo', LN, Wp'' head.

Pipeline notes (v2):
  - token stream is read twice: bf16 natural layout (mix matmul rhs) and f8
    transposed per-super-contiguous layout (score matmul stationary).
  - per-token sum(x^2) is split across ACT (4/8), DVE (2/8), GPSIMD (2/8).
  - PE emission is software-pipelined one super: scores(s) ; mix(s-1), so the
    DVE score->qp chain of super s-1 overlaps the score matmuls of super s.
  - the per-core tail runs twice on batch halves (0-3 after batch 3, 4-7 at
    the end) so most of it hides under the second half's token streaming.
"""

import numpy as np

B, S, N, D, H, C = 64, 4, 4096, 1024, 16, 14
HD = D // H
EPS = 1e-5
NCORES = 8
BLOC = B // NCORES          # batches per core
NREAL = 1 + S + N           # 4101 real tokens
NTAIL = 8                   # tail mini-super: 5 real + 3 zero-pad tokens
LTOT = N + NTAIL            # 4104 tokens kept per batch
NSUPER = 4                  # full supers of 1024 tokens (the patches region)
SUPER = 1024
NSUB = 8                    # 128-token subtiles per full super

_cache = {}


def _f32(x):
    return np.ascontiguousarray(np.asarray(x, dtype=np.float32))


def _host_prep(inputs):
    """All weight folding + token layout prep on the host (numpy)."""
    import ml_dtypes

    bf16 = ml_dtypes.bfloat16
    f8 = ml_dtypes.float8_e4m3

    cls_tok = _f32(inputs["cls_tok"])        # [B, D]
    storage = _f32(inputs["storage"])        # [B, S, D]
    patches = _f32(inputs["patches"])        # [B, N, D]
    query = _f32(inputs["query"]).reshape(D)
    g_kv = _f32(inputs["ln_kv_g"])
    b_kv = _f32(inputs["ln_kv_b"])
    Wq = _f32(inputs["Wq"]); Wk = _f32(inputs["Wk"]); Wv = _f32(inputs["Wv"])
    bq = _f32(inputs["bq"])
    Wo = _f32(inputs["Wo"]); bo = _f32(inputs["bo"])
    g_out = _f32(inputs["ln_out_g"]); b_out = _f32(inputs["ln_out_b"])
    Wp = _f32(inputs["Wp"]); bp = _f32(inputs["bp"])

    # --- score weights: s[t,h] = r_t * (x_t . w''[:, h]) ----------------------
    qp = query @ Wq.T + bq                                   # [D]
    w_raw = np.einsum("hid,hi->dh", Wk.reshape(H, HD, D), qp.reshape(H, HD))
    w_raw /= np.sqrt(HD).astype(np.float32)
    wpr = w_raw * g_kv[:, None]                              # fold LN gain
    wpp = wpr - wpr.mean(0, keepdims=True)                   # fold LN mean-centering
    # device layout [128, 8, 17]: [:, c, 0:16] = wpp[128c+p, :], col 16 = ones
    wsc_dev = np.zeros((128, 8, 17), dtype=np.float32)
    wsc_dev[:, :, :16] = wpp.reshape(8, 128, 16).transpose(1, 0, 2)
    wsc_dev[:, :, 16] = 1.0
    wsc_dev = wsc_dev.astype(bf16)

    # --- Wv / Wo / Wp folds ---------------------------------------------------
    WvT = (Wv * g_kv[None, :]).T                             # [D_in, D_out]
    wvT_dev = np.ascontiguousarray(
        WvT.reshape(8, 128, D).transpose(1, 0, 2)).astype(bf16)   # [128, 8, 1024]
    woT_dev = np.ascontiguousarray(
        Wo.T.reshape(8, 128, D).transpose(1, 0, 2)).astype(bf16)  # [128, 8, 1024]
    WpT = (Wp * g_out[None, :]).T                            # [D, C]
    wpT_dev = np.ascontiguousarray(
        WpT.reshape(8, 128, C).transpose(1, 0, 2)).astype(bf16)   # [128, 8, 14]

    bo_comb = bo + Wo @ (Wv @ b_kv)                          # [D]
    bo_dev = np.ascontiguousarray(
        bo_comb.reshape(8, 128).T).astype(np.float32)        # [128, 8] = boT
    bp_comb = (bp + Wp @ b_out).reshape(C, 1).astype(np.float32)  # [14, 1]

    # --- token stream in both layouts ----------------------------------------
    # natural: order patches (0..N-1), cls, storage, pad
    tok = np.zeros((B, LTOT, D), dtype=bf16)
    tok[:, :N] = patches.astype(bf16)
    tok[:, N] = cls_tok.astype(bf16)
    tok[:, N + 1:N + 1 + S] = storage.astype(bf16)
    # natural layout: tokens of each super grouped so partition p's rows for
    # subtiles (2m, 2m+1) are adjacent in HBM -> 4 KB DMA descriptors (2 KB
    # costs ~5% bandwidth in per-descriptor overhead; 16 KB p-major bursts
    # stall engine SBUF reads, so pairs are the sweet spot)
    #   tok_pair[b, s, m, p, jj, d] = tok[b, 1024 s + 256 m + 128 jj + p, d]
    # (pairs per partition are contiguous -> 4 KB descriptors, but different
    # partitions' pairs interleave, unlike a full p-major layout)
    tok_pair = np.ascontiguousarray(
        tok[:, :N].reshape(B, NSUPER, 4, 2, 128, D).transpose(0, 1, 2, 4, 3, 5))
    # transposed f8, p-major per-super contiguous:
    #   tokT_main[b, s, p, c, t] = tok[b, 1024 s + t, 128 c + p]
    tokf8 = tok[:, :N].astype(f8)                            # [B, N, D]
    tokT_main = np.ascontiguousarray(
        tokf8.reshape(B, NSUPER, SUPER, 8, 128).transpose(0, 1, 4, 3, 2))
    #   tokT_tail[b, p, c, t] = tok[b, N + t, 128 c + p]
    tokT_tail = np.ascontiguousarray(
        tok[:, N:].astype(f8).reshape(B, NTAIL, 8, 128).transpose(0, 3, 2, 1))

    weights = dict(wsc=wsc_dev, wvT=wvT_dev, woT=woT_dev, wpT=wpT_dev,
                   bo=bo_dev, bp=bp_comb)
    return tok_pair, np.ascontiguousarray(tok[:, N:]), tokT_main, tokT_tail, weights


def _emit(tc, io):
    """Emit the Tile program for one core (BLOC batches)."""
    import concourse.bass as bass
    from concourse import mybir

    nc = tc.nc
    f32 = mybir.dt.float32
    bf16 = mybir.dt.bfloat16
    f8 = mybir.dt.float8e4
    AF = mybir.ActivationFunctionType
    OP = mybir.AluOpType

    toknatm, toknatt, tokTm, tokTt, wsc, wvT, woT, wpT, bo, bp, out = (
        io["toknatm"], io["toknatt"], io["tokTm"], io["tokTt"], io["wsc"],
        io["wvT"], io["woT"], io["wpT"], io["bo"], io["bp"], io["out"])

    from contextlib import ExitStack
    ctx = ExitStack()
    with ctx:
        singles = ctx.enter_context(tc.tile_pool(name="singles", bufs=1))
        nat_pool = ctx.enter_context(tc.tile_pool(name="nat", bufs=6))
        tt_pool = ctx.enter_context(tc.tile_pool(name="tt", bufs=5))
        tl_pool = ctx.enter_context(tc.tile_pool(name="tl", bufs=2))
        small = ctx.enter_context(tc.tile_pool(name="small", bufs=3))
        ep_pool = ctx.enter_context(tc.tile_pool(name="ep", bufs=2))
        ps_small = ctx.enter_context(tc.tile_pool(name="ps_small", bufs=2, space="PSUM"))
        ps_mix = ctx.enter_context(tc.tile_pool(name="ps_mix", bufs=2, space="PSUM"))
        ps_den = ctx.enter_context(tc.tile_pool(name="ps_den", bufs=1, space="PSUM"))
        ps_tp = ctx.enter_context(tc.tile_pool(name="ps_tp", bufs=1, space="PSUM"))

        # ---- score weights (needed from the first super) ----------------
        wsc_sb = singles.tile([128, 8, 17], bf16)
        nc.sync.dma_start(wsc_sb[:], wsc[:])
        # tail weights are DMA'd once batch 0's token loads are queued
        wvT_sb = singles.tile([128, 8, D], bf16)
        woT_sb = singles.tile([128, 8, D], bf16)
        wpT_sb = singles.tile([128, 8, C], bf16)
        bo_sb = singles.tile([128, 8], f32)      # boT[p, k2] = bo_comb[128*k2+p]
        bp_sb = singles.tile([C, 1], f32)

        from concourse.masks import make_identity
        ident_b = singles.tile([128, 128], bf16)
        make_identity(nc, ident_b[:])
        onesf = singles.tile([128, 1], f32)
        nc.vector.memset(onesf[:], 1.0)
        ones_row = singles.tile([1, 128], f32)
        nc.vector.memset(ones_row[:], 1.0)

        # persistent per-core accumulators
        mixnT_all = singles.tile([128, 8, H, BLOC], bf16)    # [dp, c, h, b]

        x2junk = singles.tile([128, 1024], bf16)
        x2junk_a = singles.tile([128, 1024], bf16)

        def load_super(b, s):
            """Issue DMAs + x^2 + scores for (b, s); return state dict."""
            if s < NSUPER:
                natT = nat_pool.tile([128, NSUB, 1024], bf16, tag="nat")
                aux = nat_pool.tile([128, NSUB, 3], bf16, tag="aux")
                ttT = tt_pool.tile([128, 8, SUPER], f8, tag="tt")
                # single HWDGE ring: FIFO order auto-balances the two streams
                # (SWDGE descriptor-ring fetches contend with data on the SBUF
                # AXI ports and cost ~10% aggregate DMA bandwidth)
                nc.sync.dma_start(ttT[:], tokTm[b, s])
                for mq in range(4):
                    nc.sync.dma_start(natT[:, 2 * mq:2 * mq + 2, :],
                                      toknatm[b, s, mq])
                nc.vector.memset(aux[:, :, 0:1], 1.0)
                nsub, npart = NSUB, 128
            else:
                natT = tl_pool.tile([NTAIL, 1, 1024], bf16, tag="natl")
                aux = tl_pool.tile([NTAIL, 1, 3], bf16, tag="auxl")
                ttT = tl_pool.tile([128, 8, NTAIL], f8, tag="ttl")
                nc.sync.dma_start(ttT[:], tokTt[b])
                nc.sync.dma_start(
                    natT[:],
                    toknatt[b].rearrange("(j p) d -> p j d", p=NTAIL))
                nc.vector.memset(aux[:, :, 0:1], 1.0)
                nsub, npart = 1, NTAIL

            # scores: s~[tok, h] (+ col16 = sum_d x) accumulated over 8 D-chunks
            scps = ps_small.tile([128, 8, 17], f32, tag="scps")
            ntok = 128 if s < NSUPER else NTAIL
            for j in range(nsub):
                for c in range(8):
                    nc.tensor.matmul(
                        scps[0:npart, j, :],
                        lhsT=ttT[:, c, 128 * j:128 * j + ntok],
                        rhs=wsc_sb[:, c, :],
                        start=(c == 0), stop=(c == 7))
            return dict(natT=natT, aux=aux, ttT=ttT, scps=scps,
                        nsub=nsub, npart=npart, s=s, b=b)

        def x2_super(st):
            """Per-token sum of squares, split DVE 3/8 + ACT 5/8.

            Emitted AFTER chain(prev) so that on DVE/ACT the previous super's
            chain never queues behind this super's (DMA-gated) squares."""
            nsub, npart, s, natT = st["nsub"], st["npart"], st["s"], st["natT"]
            x2acc = small.tile([128, 8], f32, tag="x2acc")
            for j in range(nsub):
                if s == NSUPER:
                    src = natT[:, j, 0:1024]     # [8, 1024]
                    nc.vector.scalar_tensor_tensor(
                        out=x2junk[0:npart, :], in0=src, scalar=1.0, in1=src,
                        op0=OP.mult, op1=OP.mult,
                        accum_out=x2acc[0:npart, j:j + 1])
                elif j in (0, 3, 6):
                    nc.vector.scalar_tensor_tensor(
                        out=x2junk[:], in0=natT[:, j, 0:1024], scalar=1.0,
                        in1=natT[:, j, 0:1024], op0=OP.mult, op1=OP.mult,
                        accum_out=x2acc[:, j:j + 1])
                else:
                    nc.scalar.activation(
                        x2junk_a[:], natT[:, j, 0:1024], AF.Square,
                        accum_out=x2acc[:, j:j + 1])
            st["x2acc"] = x2acc

        def chain_super(st):
            """LN stats -> r -> softmax weights for a loaded super (DVE/ACT)."""
            nsub, npart = st["nsub"], st["npart"]
            natT, aux, x2acc, scps = (st["natT"], st["aux"], st["x2acc"],
                                      st["scps"])
            sx = small.tile([128, 8], f32, tag="sx")
            nc.vector.tensor_copy(sx[0:npart, 0:nsub], scps[0:npart, 0:nsub, 16])
            # stash mu_t = sum(x)/1024 into aux column 1 (den matmul rhs)
            nc.vector.tensor_scalar_mul(aux[:, 0:nsub, 1:2],
                                        sx[0:npart, 0:nsub], 1.0 / 1024.0)
            v0 = small.tile([128, 8], f32, tag="v0")
            nc.vector.scalar_tensor_tensor(
                out=v0[0:npart, 0:nsub],
                in0=sx[0:npart, 0:nsub], scalar=-1.0 / (1024.0 * 1024.0),
                in1=sx[0:npart, 0:nsub], op0=OP.mult, op1=OP.mult)
            nc.vector.scalar_tensor_tensor(
                out=v0[0:npart, 0:nsub],
                in0=x2acc[0:npart, 0:nsub], scalar=1.0 / 1024.0,
                in1=v0[0:npart, 0:nsub], op0=OP.mult, op1=OP.add)
            nc.vector.tensor_scalar_add(v0[0:npart, 0:nsub],
                                        v0[0:npart, 0:nsub], EPS)
            # r = rsqrt(wn) via 2 Newton steps from seed 1.0 (wn ~= 1 +- 0.07)
            rr = small.tile([128, 8], f32, tag="rr")
            ra = small.tile([128, 8], f32, tag="ra")
            rc = small.tile([128, 8], f32, tag="rc")
            nc.vector.tensor_scalar(rr[0:npart, 0:nsub], v0[0:npart, 0:nsub],
                                    -0.5, 1.5, op0=OP.mult, op1=OP.add)
            nc.vector.scalar_tensor_tensor(
                out=ra[0:npart, 0:nsub], in0=rr[0:npart, 0:nsub], scalar=1.0,
                in1=rr[0:npart, 0:nsub], op0=OP.mult, op1=OP.mult)
            nc.vector.scalar_tensor_tensor(
                out=rc[0:npart, 0:nsub], in0=v0[0:npart, 0:nsub], scalar=-0.5,
                in1=ra[0:npart, 0:nsub], op0=OP.mult, op1=OP.mult)
            nc.vector.scalar_tensor_tensor(
                out=rr[0:npart, 0:nsub], in0=rc[0:npart, 0:nsub], scalar=1.5,
                in1=rr[0:npart, 0:nsub], op0=OP.add, op1=OP.mult)
            # stash 1/r = wn * r into aux column 2: the den matmul then
            # accumulates sum_t q/r = sum_t p from the q-columns alone
            nc.vector.scalar_tensor_tensor(
                out=aux[:, 0:nsub, 2:3], in0=v0[0:npart, 0:nsub],
                scalar=1.0, in1=rr[0:npart, 0:nsub], op0=OP.mult, op1=OP.mult)

            # s = s~ * r ; p = exp(s) ; q = p * r
            s_sb = small.tile([128, 8, H], f32, tag="s")
            nc.vector.scalar_tensor_tensor(
                out=s_sb[0:npart, 0:nsub, :],
                in0=scps[0:npart, 0:nsub, 0:16], scalar=1.0,
                in1=rr[0:npart, 0:nsub, None].broadcast_to([npart, nsub, H]),
                op0=OP.mult, op1=OP.mult)
            qp_sb = small.tile([128, 8, 2 * H], bf16, tag="qp")
            if st["s"] == NSUPER:
                # tail super: zero all q/p rows, then fill the real tokens
                np_ = NREAL - N                          # rows < np_ are real
                nc.vector.memset(qp_sb[0:npart, 0:nsub, :], 0.0)
                nc.scalar.activation(qp_sb[0:np_, 0:nsub, 16:32],
                                     s_sb[0:np_, 0:nsub, :], AF.Exp)
                nc.vector.scalar_tensor_tensor(
                    out=qp_sb[0:np_, 0:nsub, 0:16],
                    in0=qp_sb[0:np_, 0:nsub, 16:32], scalar=1.0,
                    in1=rr[0:np_, 0:nsub, None].broadcast_to([np_, nsub, H]),
                    op0=OP.mult, op1=OP.mult)
            else:
                nc.scalar.activation(qp_sb[:, 0:nsub, 16:32],
                                     s_sb[:, 0:nsub, :], AF.Exp)
                nc.vector.scalar_tensor_tensor(
                    out=qp_sb[:, 0:nsub, 0:16],
                    in0=qp_sb[:, 0:nsub, 16:32], scalar=1.0,
                    in1=rr[:, 0:nsub, None].broadcast_to([128, nsub, H]),
                    op0=OP.mult, op1=OP.mult)
            st["qp"] = qp_sb

        def mix_super(st, mixps, denps, first_mm, last_s):
            """Accumulate mix/den matmuls for a chained super.

            One 16-col stationary (the q columns) serves all three matmuls:
            denps cols are [sum q | sum q*mu | sum q/r] = [junk | c1 | den]."""
            nsub, npart = st["nsub"], st["npart"]
            natT, aux, qp_sb = st["natT"], st["aux"], st["qp"]
            for j in range(nsub):
                last = last_s and (j == nsub - 1)
                nc.tensor.matmul(
                    mixps[:, 0:512], lhsT=qp_sb[0:npart, j, 0:16],
                    rhs=natT[0:npart, j, 0:512], start=first_mm, stop=last)
                nc.tensor.matmul(
                    mixps[:, 512:1024], lhsT=qp_sb[0:npart, j, 0:16],
                    rhs=natT[0:npart, j, 512:1024], start=first_mm, stop=last)
                nc.tensor.matmul(
                    denps[:, 0:3], lhsT=qp_sb[0:npart, j, 0:16],
                    rhs=aux[0:npart, j, 0:3], start=first_mm, stop=last)
                first_mm = False

        def epilogue_batch(b, mixps, denps):
            """Per-batch: head mix -> mixnT_all[:, :, :, b]."""
            dinv = ep_pool.tile([H, 1], f32, tag="dinv")
            nc.vector.reciprocal(dinv[:], denps[:, 2:3])
            c1 = ep_pool.tile([H, 1], f32, tag="c1")
            nc.vector.tensor_copy(c1[:], denps[:, 1:2])
            mixn = ep_pool.tile([H, D], bf16, tag="mixn")
            nc.vector.scalar_tensor_tensor(
                out=mixn[:], in0=mixps[:], scalar=c1[:],
                in1=dinv[:, 0:1].broadcast_to([H, D]),
                op0=OP.subtract, op1=OP.mult)
            tp_all = ps_tp.tile([128, 8, H], bf16, tag="tp")
            for c in range(8):
                nc.tensor.transpose(tp_all[:, c, :], mixn[:, 128 * c:128 * c + 128],
                                    ident_b[0:H, 0:H])
            nc.vector.tensor_copy(mixnT_all[:, :, :, b], tp_all[:])

        def tail_half(b0, qpools):
            """Per-core tail for batches [b0, b0+4): Wv/Wo matmuls, out-LN, head.

            qpools: (pool, tag) per psum tile in allocation order
            [ctxps, poolps, sums, bcast, ops_] — each entry must be a slot
            that is provably free during this tail (see call sites)."""
            BL = 4
            qi = [0]

            def qtile(shape, dt_, nm):
                pool, tg = qpools[qi[0]]
                qi[0] += 1
                return pool.tile(shape, dt_, tag=tg, name=f"tl{b0}_{nm}")

            ctxT_sb = singles.tile([128, 8, BLOC], bf16, tag="ctxT")
            poolT_sb = singles.tile([128, 8, BL], f32, tag="poolT")
            sq_sb = singles.tile([128, 8, BL], f32, tag="sq")
            # ctxT[o, b] = sum_d Wv'[o, d] mixn[head(o), d]  (block-diag heads)
            ctxps = qtile([128, 8, BL], f32, "ctxps")
            for k in range(8):
                for half in range(2):
                    h = 2 * k + half
                    for c in range(8):
                        nc.tensor.matmul(
                            ctxps[64 * half:64 * half + 64, k, :],
                            lhsT=wvT_sb[:, c, 64 * h:64 * h + 64],
                            rhs=mixnT_all[:, c, h, b0:b0 + BL],
                            start=(c == 0), stop=(c == 7))
            nc.vector.tensor_copy(ctxT_sb[:, :, b0:b0 + BL], ctxps[:])
            # pooledT[o2, b] = sum_o Wo[o2, o] ctx[o, b] + boT
            poolps = qtile([128, 8, BL], f32, "poolps")
            for k2 in range(8):
                for k in range(8):
                    nc.tensor.matmul(
                        poolps[:, k2, :],
                        lhsT=woT_sb[:, k, 128 * k2:128 * k2 + 128],
                        rhs=ctxT_sb[:, k, b0:b0 + BL],
                        start=(k == 0), stop=(k == 7))
            nc.vector.scalar_tensor_tensor(
                out=poolT_sb[:], in0=poolps[:], scalar=1.0,
                in1=bo_sb[:, :, None].broadcast_to([128, 8, BL]),
                op0=OP.mult, op1=OP.add)
            nc.scalar.square(sq_sb[:], poolT_sb[:])
            # LN stats over the o2 (partition+chunk) axis via ones-matmuls
            sums = qtile([1, 2 * BL], f32, "sums")
            for k2 in range(8):
                nc.tensor.matmul(sums[0:1, 0:BL], lhsT=onesf[:, 0:1],
                                 rhs=poolT_sb[:, k2, :],
                                 start=(k2 == 0), stop=(k2 == 7))
            for k2 in range(8):
                nc.tensor.matmul(sums[0:1, BL:2 * BL], lhsT=onesf[:, 0:1],
                                 rhs=sq_sb[:, k2, :],
                                 start=(k2 == 0), stop=(k2 == 7))
            stats = singles.tile([1, 2 * BL], f32, tag="stats")
            nc.vector.tensor_copy(stats[:], sums[:])
            v8 = singles.tile([1, BL], f32, tag="v8")
            nc.vector.scalar_tensor_tensor(
                out=v8[:], in0=stats[0:1, 0:BL],
                scalar=-1.0 / (1024.0 * 1024.0),
                in1=stats[0:1, 0:BL], op0=OP.mult, op1=OP.mult)
            nc.vector.scalar_tensor_tensor(
                out=v8[:], in0=stats[0:1, BL:2 * BL], scalar=1.0 / 1024.0,
                in1=v8[:], op0=OP.mult, op1=OP.add)
            nc.vector.tensor_scalar_add(v8[:], v8[:], EPS)
            r8 = singles.tile([1, BL], f32, tag="r8")
            nc.vector.reciprocal(r8[:], v8[:])
            nc.scalar.sqrt(r8[:], r8[:])                     # rsqrt(var+eps)
            pair = singles.tile([1, 2 * BL], f32, tag="pair")  # [-mu*r | r]
            nc.vector.scalar_tensor_tensor(
                out=pair[0:1, 0:BL], in0=stats[0:1, 0:BL], scalar=-1.0 / 1024.0,
                in1=r8[:], op0=OP.mult, op1=OP.mult)
            nc.vector.tensor_copy(pair[0:1, BL:2 * BL], r8[:])
            bcast = qtile([128, 2 * BL], f32, "bcast")
            nc.tensor.matmul(bcast[:], lhsT=ones_row[0:1, :], rhs=pair[:],
                             start=True, stop=True)
            nr_bc = singles.tile([128, 1, 2 * BL], f32, tag="nrbc")
            nc.vector.tensor_copy(nr_bc[:, 0, :], bcast[:])

            # yhatT = (poolT - mu) * r in [o2, b] layout, then the head matmul
            yhatT = singles.tile([128, 8, BL], bf16, tag="yhat")
            tn = singles.tile([128, 8, BL], f32, tag="tn")
            nc.vector.scalar_tensor_tensor(
                out=tn[:], in0=poolT_sb[:], scalar=1.0,
                in1=nr_bc[:, 0:1, BL:2 * BL].broadcast_to([128, 8, BL]),
                op0=OP.mult, op1=OP.mult)
            nc.vector.scalar_tensor_tensor(
                out=yhatT[:], in0=tn[:], scalar=1.0,
                in1=nr_bc[:, 0:1, 0:BL].broadcast_to([128, 8, BL]),
                op0=OP.mult, op1=OP.add)
            ops_ = qtile([C, BL], f32, "ops")
            for c in range(8):
                nc.tensor.matmul(ops_[:], lhsT=wpT_sb[:, c, :],
                                 rhs=yhatT[:, c, :],
                                 start=(c == 0), stop=(c == 7))
            out_sb = singles.tile([C, BL], f32, tag="outsb")
            nc.vector.tensor_scalar(out_sb[:], ops_[:], bp_sb[:], None,
                                    op0=OP.add)
            nc.sync.dma_start(out[b0:b0 + BL, :].rearrange("b c -> c b"),
                              out_sb[:])

        # ================= main software-pipelined loop ===================
        # stream of (b, s) supers; PE does scores(i) then mix(i-1)
        stream = [(b, s) for b in range(BLOC) for s in range(NSUPER + 1)]
        SPB = NSUPER + 1                     # supers per batch
        prev = None                          # state of super i-1 (loaded)
        mixq = {}                            # batch -> [mixps, denps, first_mm]

        for idx, (b, s) in enumerate(stream):
            if s == 0:
                mixq[b] = [ps_mix.tile([H, D], f32, tag="mix", name=f"mixps{b}"),
                           ps_den.tile([H, 3], f32, tag="den", name=f"denps{b}"),
                           True]
            st = load_super(b, s)            # DMA + scores for (b, s)
            if prev is not None:
                chain_super(prev)            # DVE/ACT chain for i-1
                pb = prev["b"]
                m = mixq[pb]
                mix_super(prev, m[0], m[1], m[2], last_s=(prev["s"] == NSUPER))
                m[2] = False
            x2_super(st)                     # DVE/ACT squares for (b, s)
            if prev is not None:
                pb = prev["b"]
                if prev["s"] == NSUPER:      # batch pb fully accumulated
                    epilogue_batch(pb, m[0], m[1])
                    del mixq[pb]
                    if pb == 3:
                        # batch 3's mix slot was just released; batch 5's
                        # alloc comes later, so the tail may borrow it ONCE.
                        # Everything else funnels through the tp slot (the
                        # other mix slot and both scps slots are live, and
                        # borrowing them would deadlock DVE program order).
                        tail_half(0, [(ps_tp, "tp"), (ps_mix, "mix"),
                                      (ps_tp, "tp"), (ps_tp, "tp"),
                                      (ps_tp, "tp")])
            prev = st
            if idx == SPB + 2:
                # tail weights queued mid-batch-1: the prefetch window has
                # enough slack there to absorb their ~12us of ring time
                nc.sync.dma_start(wvT_sb[:], wvT[:])
                nc.sync.dma_start(woT_sb[:], woT[:])
                nc.sync.dma_start(wpT_sb[:], wpT[:])
                nc.sync.dma_start(bo_sb[:], bo[:])
                nc.sync.dma_start(bp_sb[:], bp[:])

        # drain the last super + last batch epilogue + second tail half
        chain_super(prev)
        m = mixq[prev["b"]]
        mix_super(prev, m[0], m[1], m[2], last_s=True)
        epilogue_batch(prev["b"], m[0], m[1])
        # the whole main loop is drained: every psum pool is free for the tail
        tail_half(4, [(ps_tp, "tp"), (ps_mix, "mix"), (ps_small, "scps"),
                      (ps_mix, "mix"), (ps_den, "den")])


def _build():
    import concourse.bass as bass
    import concourse.bacc as bacc
    import concourse.tile as tile
    from concourse import mybir

    f32 = mybir.dt.float32
    bf16 = mybir.dt.bfloat16

    nc = bacc.Bacc("TRN2", target_bir_lowering=False, debug=False,
                   num_devices=NCORES)
    io = {
        "toknatm": nc.dram_tensor("toknatm", [BLOC, NSUPER, 4, 128, 2, D],
                                  bf16, kind="ExternalInput").ap(),
        "toknatt": nc.dram_tensor("toknatt", [BLOC, NTAIL, D], bf16,
                                  kind="ExternalInput").ap(),
        "tokTm": nc.dram_tensor("tokTm", [BLOC, NSUPER, 128, 8, SUPER],
                                mybir.dt.float8e4, kind="ExternalInput").ap(),
        "tokTt": nc.dram_tensor("tokTt", [BLOC, 128, 8, NTAIL],
                                mybir.dt.float8e4, kind="ExternalInput").ap(),
        "wsc": nc.dram_tensor("wsc", [128, 8, 17], bf16,
                              kind="ExternalInput").ap(),
        "wvT": nc.dram_tensor("wvT", [128, 8, D], bf16,
                              kind="ExternalInput").ap(),
        "woT": nc.dram_tensor("woT", [128, 8, D], bf16,
                              kind="ExternalInput").ap(),
        "wpT": nc.dram_tensor("wpT", [128, 8, C], bf16,
                              kind="ExternalInput").ap(),
        "bo": nc.dram_tensor("bo", [128, 8], f32, kind="ExternalInput").ap(),
        "bp": nc.dram_tensor("bp", [C, 1], f32, kind="ExternalInput").ap(),
        "out": nc.dram_tensor("out", [BLOC, C], f32,
                              kind="ExternalOutput").ap(),
    }
    with tile.TileContext(nc) as tc:
        _emit(tc, io)
    nc.compile()
    return nc


def _get_nc():
    if "nc" not in _cache:
        _cache["nc"] = _build()
    return _cache["nc"]


def run(inputs, trace=False, trace_kwargs=None):
    """Shard, run on 8 cores, gather.  Returns (out, BassKernelResults)."""
    from concourse.bass_utils import run_bass_kernel_spmd

    tokm, tokt, tokTm, tokTt, weights = _host_prep(inputs)
    nc = _get_nc()
    in_maps = []
    for i in range(NCORES):
        m = dict(weights)
        m["toknatm"] = np.ascontiguousarray(tokm[i * BLOC:(i + 1) * BLOC])
        m["toknatt"] = np.ascontiguousarray(tokt[i * BLOC:(i + 1) * BLOC])
        m["tokTm"] = np.ascontiguousarray(tokTm[i * BLOC:(i + 1) * BLOC])
        m["tokTt"] = np.ascontiguousarray(tokTt[i * BLOC:(i + 1) * BLOC])
        in_maps.append(m)
    res = run_bass_kernel_spmd(nc, in_maps, core_ids=list(range(NCORES)),
                               trace=trace, **(trace_kwargs or {}))
    out = np.concatenate([np.asarray(res.results[i]["out"], dtype=np.float32)
                          for i in range(NCORES)], axis=0)
    return out, res


def kernel(**inputs):
    out, _ = run(inputs)
    return out
